# revision 50
# baseline (speedup 1.0000x reference)
"""Multi-head attention (B=2, S=2048, D=1024, H=16, d_k=64) on 8 TRN2 NeuronCores.

Sharding: head-parallel. Core c owns heads (2c, 2c+1) for both batch rows:
 - replicated inputs: qT/kT/vT host-packed as per-(batch, 512-col j-block)
   blocks [128, ND*512] so each block is ONE [128, 4096] DMA with an 8KB
   contiguous run per partition (full hardware-DGE rate, cheap trigger);
   within a block the 8 d-chunks are column-major groups, d on partitions
   so the TensorEngine contracts over D with no transposes.
 - per-core weights: Wq columns / Wo rows for its two heads (host pre-packs
   wq/wqv d-major so each is ONE contiguous DMA)
 - per-core output: partial = attn_out(own heads) @ Wo[own rows]  [4096, 1024] bf16
   The host sums the 8 partials (f32) and adds bo.  No cross-core comm.

Per-core dataflow (bf16 matmuls, f32 PSUM):
 1. wq + bq + all x blocks ride the sync hardware-DMA queue (the gpsimd
    software queue starts ~10us late and runs ~3x slower -- only the
    late-needed consts and b0 stores go there).  Sync order: q(b0,j0),
    k(b0,j0..3), q(b0,j1..3), v(b0,j0..3), then b1 q/k/v via lane-A
    thunks.  qh-j0 projects as soon as its block lands; kh j0..3
    projections chase their DMAs as lane-A items inside the t-loop, so
    scores(sc0) start ~7us after the first matmul instead of waiting for
    all of k.
 2. qhT/khT [128, 2048] per batch = Wq_c.T @ xT (+bq).  vh [2048, 130]
    natural = vT.T @ Wqv_c; Wqv has zero-cols / bqv has 1.0-cols so each
    head gets a ones column -> attn@V also produces softmax denominators.
 3. scoresT[t,s] = khT.T @ qhT, both heads packed into disjoint PE
    row-groups (K=64).  exp(x/8) on ScalarE from PSUM, bf16 out.
 4. attn@V accumulated over t; row 64 = denominator.  Normalize: DVE
    fast-reciprocal straight off the two denominator rows into a [1,2,512]
    f32 tile, bf16 rounding copy, per-head K=1 matmuls against the sel2
    ones row broadcast the reciprocals across partitions, then DVE muls
    (att sbuf x bcast psum) produce normalized oT bf16.
 5. partial[s, :] = outT.T @ Wo_c -> bf16 ob [128,1024] (two PSUM copies)
    -> ONE DRAM store per 128 rows.  b0 stores ride gpsimd; b1 stores ride
    sync (free after the b1 loads), so the tail drain is short.

Scheduling: ScalarE (exp, ~143us) and the TensorEngine (~170us execute) --
PE is the bottleneck, so emission keeps the PE queue dense: each s-chunk's
scores+exp loop is emitted first; its attn@V/normalize/out-proj are
deferred one s-chunk and re-emitted between later score iterations via a
three-lane work queue (lane A: DMA-gated projection work with
earliest-iteration thresholds; lane B: deferred attention work, also
min-iter gated so no PE instruction is emitted before its input DMA can
have landed -- the PE queue is in-order, a stalled instruction blocks it).
"""

import numpy as np
import ml_dtypes

B, S, D, H, DK = 2, 2048, 1024, 16, 64
NCORES = 8
HPC = H // NCORES          # heads per core = 2
BS = B * S                 # 4096
HD = HPC * DK              # 128 = per-core head dims
ND = D // 128              # 8 d-chunks
NSC = S // 512             # 4 column blocks per batch
BLK = ND * 512             # 4096 elems per partition per block

_cache = {}


def _build():
    import concourse.bass as bass
    import concourse.tile as tile
    from concourse import bacc, mybir

    f32 = mybir.dt.float32
    bf16 = mybir.dt.bfloat16
    Exp = mybir.ActivationFunctionType.Exp

    nc = bacc.Bacc("TRN2", target_bir_lowering=False, debug=False,
                   num_devices=NCORES)

    qT = nc.declare_dram_parameter("qT", [128, B * NSC * BLK], bf16,
                                   isOutput=False)
    kT = nc.declare_dram_parameter("kT", [128, B * NSC * BLK], bf16,
                                   isOutput=False)
    vT = nc.declare_dram_parameter("vT", [128, B * NSC * BLK], bf16,
                                   isOutput=False)
    wq = nc.declare_dram_parameter("wq", [128, ND * HD], bf16, isOutput=False)
    wqv = nc.declare_dram_parameter("wqv", [128, ND * 130], bf16, isOutput=False)
    bqc = nc.declare_dram_parameter("bqc", [HD, 1], f32, isOutput=False)
    bqvb = nc.declare_dram_parameter("bqvb", [128, 130], f32, isOutput=False)
    wo = nc.declare_dram_parameter("wo", [HD, D], bf16, isOutput=False)
    sel2d = nc.declare_dram_parameter("sel2", [2, 128], bf16, isOutput=False)
    out = nc.declare_dram_parameter("out", [BS, D], bf16, isOutput=True)

    NT = S // 128            # 16 t-chunks per batch

    with tile.TileContext(nc) as tc:
        with (
            tc.tile_pool(name="const", bufs=1) as pc,
            tc.tile_pool(name="xg", bufs=12) as pin1,
            tc.tile_pool(name="proj", bufs=2) as pproj,
            tc.tile_pool(name="vh", bufs=2) as pvh,
            tc.tile_pool(name="exp", bufs=19) as pexp,
            tc.tile_pool(name="outT", bufs=2) as poutT,
            tc.tile_pool(name="small", bufs=2) as psmall,
            tc.tile_pool(name="ob", bufs=3) as pob,
            tc.tile_pool(name="ps", bufs=2, space="PSUM") as pps,
        ):
            # ---- sync hardware queue: wq + bq first (first proj needs
            # them), then q j0, k j0..3, q j1..3, v j0..3
            wq_sb = pc.tile([128, ND * HD], bf16)
            nc.sync.dma_start(wq_sb[:], wq[:, :])
            bq_col = pc.tile([128, 1], f32)
            nc.sync.dma_start(bq_col[:], bqc[:, :])

            def dma_blk(src, b, j, nm):
                t = pin1.tile([128, BLK], bf16, tag="xg",
                              name=f"x{nm}{b}{j}")
                off = (b * NSC + j) * BLK
                nc.sync.dma_start(t[:], src[:, off:off + BLK])
                return t

            qx0 = [None] * NSC
            kx0 = [None] * NSC
            vx0 = [None] * NSC
            qx0[0] = dma_blk(qT, 0, 0, "q")
            for j in range(NSC):
                kx0[j] = dma_blk(kT, 0, j, "k")
            for j in range(1, NSC):
                qx0[j] = dma_blk(qT, 0, j, "q")
            for j in range(NSC):
                vx0[j] = dma_blk(vT, 0, j, "v")

            # ---- gpsimd software queue: only late-needed consts
            bqv_bc = pc.tile([128, 130], f32)
            nc.gpsimd.dma_start(bqv_bc[:], bqvb[:, :])
            wqv_sb = pc.tile([128, ND * 130], bf16)
            nc.gpsimd.dma_start(wqv_sb[:], wqv[:, :])
            wo_sb = pc.tile([HD, D], bf16)
            nc.gpsimd.dma_start(wo_sb[:], wo[:, :])
            # [2,128] bf16 selector (host-packed): row 0 is ones on cols
            # 0:64 (the K=1 broadcast stationary); row h is 1 on head h's
            # 64 cols
            sel2 = pc.tile([2, 128], bf16)
            nc.gpsimd.dma_start(sel2[:], sel2d[:, :])


            def wqd(d):
                return wq_sb[:, d * HD:(d + 1) * HD]

            def wqvd(d):
                return wqv_sb[:, d * 130:(d + 1) * 130]

            qh0 = pproj.tile([128, S], bf16, tag="projq", name="projq0")
            kh0 = pproj.tile([128, S], bf16, tag="projk", name="projk0")

            # one j-block projection: 8 accumulating matmuls + bias add
            def proj_j(xt, sb, j):
                ps = pps.tile([128, 512], f32, tag="p1", name=f"pj{j}")
                for d in range(ND):
                    nc.tensor.matmul(ps, wqd(d),
                                     xt[:, d * 512:(d + 1) * 512],
                                     start=(d == 0), stop=(d == ND - 1))
                nc.vector.tensor_scalar_add(
                    sb[:, j * 512:(j + 1) * 512], ps, bq_col[:])

            # qh j0 + kh j0 before the t-loop; kh j1..3 chase their DMAs
            # as lane-A items inside it.
            proj_j(qx0[0], qh0, 0)
            proj_j(kx0[0], kh0, 0)

            # ---- three-lane deferred work queue ----
            laneA = []   # (min_iter, thunk): DMA-gated projection work
            laneB = []   # (min_iter, ready_fn, thunk): attn@V + asb copies
            laneC = []   # (min_iter, ready_fn, thunk): norm
            laneD = []   # (min_iter, ready_fn, thunk): out-proj, hoarded
            #              late so it fills the laneA-dry b1 second half
            it = [0]
            done = {}    # emission flags: (name, t) -> True

            def pump():
                popped = 0
                if laneA and laneA[0][0] <= it[0]:
                    laneA.pop(0)[1]()
                    popped = 1
                for _ in range(2 - popped):
                    if laneB and laneB[0][0] <= it[0] and laneB[0][1]():
                        laneB.pop(0)[2]()
                if laneC and laneC[0][0] <= it[0] and laneC[0][1]():
                    laneC.pop(0)[2]()
                if laneD and laneD[0][0] <= it[0] and laneD[0][1]():
                    laneD.pop(0)[2]()
                it[0] += 1

            hold = {}

            # vh items: [128, 130] t-chunks; block j = t//4
            def vh_item(b, t, xv_of):
                def tt():
                    ps = pps.tile([128, 130], f32, tag="p1", name=f"pvh{b}{t}")
                    for d in range(ND):
                        nc.tensor.matmul(ps, xv_of(d, t), wqvd(d),
                                         start=(d == 0), stop=(d == ND - 1))
                    nc.vector.tensor_add(hold["vh" + str(b)][:, t, :],
                                         ps[:], bqv_bc[:])
                    done[("vh" + str(b), t)] = True
                return tt

            def xv_slice(blocks, d, t):
                c = d * 512 + (t % 4) * 128
                return blocks[t // 4][:, c:c + 128]

            def xv0_of(d, t):
                return xv_slice(vx0, d, t)

            def xv1_of(d, t):
                return xv_slice(hold["vt1"], d, t)

            # b1 q/k block loads + projections as lane-A items.  Each b1
            # block DMA reuses an xg buf whose b0 reader must already be
            # EMITTED (Tile WAR deps only see emitted readers), so each
            # load is its own item gated just past that reader's item.
            def qk_chain_thunks(base, step):
                items = []

                def alloc_thunk():
                    hold["qt1"] = [None] * NSC
                    hold["kt1"] = [None] * NSC
                    hold["qh"] = pproj.tile([128, S], bf16, tag="projq",
                                            name="projq1")
                    hold["kh"] = pproj.tile([128, S], bf16, tag="projk",
                                            name="projk1")
                    hold["qt1"][0] = dma_blk(qT, 1, 0, "q")
                    hold["qt1"][1] = dma_blk(qT, 1, 1, "q")
                items.append((1, alloc_thunk))

                def b1_dma(tgt, src, j, nm):
                    def th():
                        hold[tgt][j] = dma_blk(src, 1, j, nm)
                    return th
                items.append((6, b1_dma("qt1", qT, 2, "q")))
                items.append((9, b1_dma("qt1", qT, 3, "q")))
                items.append((11, b1_dma("kt1", kT, 0, "k")))
                items.append((13, b1_dma("kt1", kT, 1, "k")))
                items.append((16, b1_dma("kt1", kT, 2, "k")))
                items.append((18, b1_dma("kt1", kT, 3, "k")))
                cell = {}
                for i, name in enumerate(("q", "k")):
                    for j in range(NSC):
                        def t1a(name=name, j=j):
                            ps = pps.tile([128, 512], f32,
                                          tag="p1", name=f"pb{name}{j}")
                            xt = hold["qt1" if name == "q" else "kt1"][j]
                            for d in range(4):
                                nc.tensor.matmul(
                                    ps, wqd(d), xt[:, d * 512:(d + 1) * 512],
                                    start=(d == 0), stop=False)
                            cell[(name, j)] = ps

                        def t1b(name=name, j=j):
                            ps = cell[(name, j)]
                            xt = hold["qt1" if name == "q" else "kt1"][j]
                            for d in range(4, ND):
                                nc.tensor.matmul(
                                    ps, wqd(d), xt[:, d * 512:(d + 1) * 512],
                                    start=False,
                                    stop=(d == ND - 1))
                            sb = hold["qh" if name == "q" else "kh"]
                            nc.vector.tensor_scalar_add(
                                sb[:, j * 512:(j + 1) * 512], ps, bq_col[:])
                        items.append((base[i] + step * j, t1a))
                        items.append((base[i] + step * j, t1b))
                return items

            f32r = mybir.dt.float32r

            def norm_rec_a(att, sfx, den_eng=None):
                # den rows copied to partition 0 first: the custom-DVE
                # reciprocal mislowers partition-offset inputs on HW.
                den = psmall.tile([1, 2, 512], f32, tag="dcp", bufs=1,
                                  name="den" + sfx)
                nc.vector.tensor_copy(den[0:1, 0, :], att[0][64:65, :])
                if den_eng is None:
                    nc.vector.tensor_copy(den[0:1, 1, :], att[1][64:65, :])
                else:
                    den_eng.copy(den[0:1, 1, :], att[1][64:65, :])
                rec = psmall.tile([1, 2, 512], f32, tag="den", bufs=1,
                                  name="rec" + sfx)
                nc.vector.reciprocal_approx_fast(rec[:], den[:])
                recb = psmall.tile([1, 2, 512], bf16, tag="recb", bufs=1,
                                   name="recb" + sfx)
                nc.vector.tensor_copy(recb[:], rec[:])
                return recb

            def norm_rec_b(recb, sfx):
                # per-head K=1 broadcast matmuls against the sel2 ones row
                bcds = []
                for h in range(HPC):
                    bcd = pps.tile([64, 512], f32, tag="p1",
                                   name=f"bcd{h}" + sfx)
                    nc.tensor.matmul(bcd, sel2[0:1, 0:64], recb[0:1, h, :],
                                     start=True, stop=True)
                    bcds.append(bcd)
                return bcds

            def norm_v2(att, oT, ssl, sfx):
                # deferred path: att is sbuf (asb); muls read bcd psum
                bcds = norm_rec_b(norm_rec_a(att, sfx), sfx)
                for h in range(HPC):
                    hp = slice(h * 64, (h + 1) * 64)
                    nc.vector.tensor_mul(oT[hp, ssl],
                                         att[h][0:64, :], bcds[h][:, :])

            def outproj_s1(b, sc, s1, oT, sfx, cast_eng=None,
                           split_store=False):
                s0 = sc * 512 + s1 * 128
                rs = slice(b * S + s0, b * S + s0 + 128)
                ob = pob.tile([128, D], bf16, tag="ob", name="ob" + sfx)
                # b0 stores ride the gpsimd software queue; b1 stores ride
                # sync (free after the b1 loads) so the tail drains fast
                seng = nc.gpsimd if b == 0 else nc.sync
                for n in range(2):
                    nsl = slice(n * 512, (n + 1) * 512)
                    ps = pps.tile([128, 512], f32, tag="p1",
                                  name="opps" + sfx)
                    nc.tensor.matmul(ps, oT[:, s0:s0 + 128], wo_sb[:, nsl],
                                     start=True, stop=True)
                    if cast_eng is None:
                        nc.vector.tensor_copy(ob[:, nsl], ps)
                    else:
                        cast_eng.copy(ob[:, nsl], ps)
                    if split_store:
                        eng = nc.gpsimd if n == 0 else nc.sync
                        eng.dma_start(out[rs, nsl], ob[:, nsl])
                if not split_store:
                    seng.dma_start(out[rs, :], ob[:])

            def defer_attnv(b, sc, exs, vh_of, oT, gate, og):
                ssl = slice(sc * 512, (sc + 1) * 512)
                cell = {}
                d0g = it[0]
                for t in range(NT):
                    def av(t=t):
                        if t == 0:
                            cell["att"] = [
                                pps.tile([65, 512], f32, tag="att",
                                         name=f"att{b}{sc}{h}")
                                for h in range(HPC)]
                        vh = vh_of()
                        for h in range(HPC):
                            nc.tensor.matmul(cell["att"][h],
                                             vh[:, t, h * 65:h * 65 + 65],
                                             exs[t][:, h, :],
                                             start=(t == 0), stop=(t == NT - 1))
                    g = gate(t) if gate is not None else 0
                    laneB.append((g,
                                  (lambda t=t: ("vh" + str(b), t) in done),
                                  av))

                # asb copies (DVE-only) free the att psum bank promptly so
                # the next s-chunk's attn@V can start; the norm matmul +
                # out-proj go to laneC, gated past the last attn@V so score
                # matmuls sit between them in the in-order PE queue, hiding
                # the DVE reciprocal-chain latency.
                def asb_copy():
                    cell["asb"] = [
                        psmall.tile([65, 512], f32, tag="asb", bufs=4,
                                    name=f"asb{b}{sc}{h}")
                        for h in range(HPC)]
                    for h in range(HPC):
                        nc.vector.tensor_copy(cell["asb"][h][:],
                                              cell["att"][h][:])
                laneB.append((0, lambda: True, asb_copy))
                d0 = it[0]

                def norm():
                    norm_v2(cell["asb"], oT, ssl, f"{b}{sc}")
                    cell["normed"] = True
                laneC.append((d0 + 13, lambda: "asb" in cell, norm))

                for g in range(2):
                    def op(g=g):
                        for u in range(2):
                            outproj_s1(b, sc, g * 2 + u, oT, f"{b}{sc}")
                    laneD.append((og[g], lambda: "normed" in cell, op))

            def inline_tail(b, sc, att, oT):
                # att is psum here.  den-h1 copy rides ScalarE (exps are
                # done; the COPY table is shared with the ob casts), bcs
                # copies keep the muls off dual-psum reads, and each
                # u-chunk's muls feed its outproj immediately.  The last
                # deferred outproj group is drained between the norm
                # phases: real PE work covering the DVE reciprocal chain.
                ssl = slice(sc * 512, (sc + 1) * 512)
                recb = norm_rec_a(att, "L", den_eng=nc.scalar)
                if laneD and laneD[0][1]():
                    laneD.pop(0)[2]()
                bcds = norm_rec_b(recb, "L")
                if laneD and laneD[0][1]():
                    laneD.pop(0)[2]()
                bcss = []
                for h in range(HPC):
                    bcs = psmall.tile([64, 512], f32, tag="bcs", bufs=1,
                                      name=f"bcs{h}L")
                    nc.vector.tensor_copy(bcs[:], bcds[h][:, :])
                    bcss.append(bcs)
                for u in range(4):
                    us = slice(u * 128, (u + 1) * 128)
                    for h in range(HPC):
                        hp = slice(h * 64, (h + 1) * 64)
                        nc.vector.tensor_mul(oT[hp, ssl][:, us],
                                             att[h][0:64, us], bcss[h][:, us])
                    eng = nc.scalar if u % 2 else None
                    outproj_s1(b, sc, u, oT, "L", cast_eng=eng,
                               split_store=(u == 3))

            def attention(b, qh, kh, vh_of, gate0=None, last=False):
                oT = poutT.tile([128, S], bf16, tag="outT", name=f"oT{b}")
                for sc in range(NSC):
                    inline = last and sc == NSC - 1
                    exs = []
                    att = None
                    for t in range(NT):
                        pump()
                        scps = pps.tile([128, HPC, 512], f32, tag="sc")
                        for h in range(HPC):
                            hp = slice(h * 64, (h + 1) * 64)
                            nc.tensor.matmul(scps[:, h, :],
                                             kh[hp, t * 128:(t + 1) * 128],
                                             qh[hp, sc * 512:(sc + 1) * 512],
                                             start=True, stop=True)
                        ex = pexp.tile([128, HPC, 512], bf16, tag="exp")
                        nc.scalar.activation(ex[:], scps[:], Exp, scale=0.125)
                        exs.append(ex)
                        if inline:
                            if att is None:
                                att = [pps.tile([65, 512], f32, tag="att",
                                                name=f"attL{h}")
                                       for h in range(HPC)]
                            vh = vh_of()
                            for h in range(HPC):
                                nc.tensor.matmul(att[h],
                                                 vh[:, t, h * 65:h * 65 + 65],
                                                 ex[:, h, :],
                                                 start=(t == 0),
                                                 stop=(t == NT - 1))
                    if inline:
                        inline_tail(b, sc, att, oT)
                    else:
                        gate = gate0 if (gate0 is not None and sc == 0) \
                            else None
                        # outproj spread through the next window; b1-sc2's
                        # goes to the inline tail's manual drain (129/130
                        # never pop in-loop)
                        idx = b * NSC + sc
                        og = (it[0] + 14, it[0] + 15) if idx < 6 \
                            else (129, 130)
                        defer_attnv(b, sc, exs, vh_of, oT, gate, og)

            # ---- lane-A schedule ----
            # iters are scores-iterations (~1.1us each from ~13us).
            # landings (sync hw queue, ramping ~170->280GB/s): k j1/2/3
            # ~18/21/23us -> kh-j at 5/8/10; q j1..3 ~26/29/31 -> qh-j at
            # 12/15/17; v0 j-blocks ~34/37/40/43 -> vh t at 19+t; b1 q1
            # ~46-55 -> 30+3j, k1 ~58-67 -> 41+3j, v1 ~70-79 -> vh1 52+t.
            hold["vh0"] = pvh.tile([128, NT, 130], bf16, tag="vh", name="vh0")
            items = [(4, lambda: proj_j(kx0[1], kh0, 1)),
                     (8, lambda: proj_j(kx0[2], kh0, 2)),
                     (10, lambda: proj_j(kx0[3], kh0, 3)),
                     (12, lambda: proj_j(qx0[1], qh0, 1)),
                     (15, lambda: proj_j(qx0[2], qh0, 2)),
                     (17, lambda: proj_j(qx0[3], qh0, 3))]
            for t in range(NT):
                items.append((19 + t, vh_item(0, t, xv0_of)))
            items += qk_chain_thunks((30, 41), 3)

            def v1_first():
                hold["vt1"] = [None] * NSC
                hold["vh1"] = pvh.tile([128, NT, 130], bf16, tag="vh",
                                       name="vh1")
                hold["vt1"][0] = dma_blk(vT, 1, 0, "v")
            items.append((23, v1_first))
            for j in range(1, NSC):
                def v1_blk(j=j):
                    hold["vt1"][j] = dma_blk(vT, 1, j, "v")
                items.append((23 + 4 * j, v1_blk))
            for t in range(NT):
                items.append((52 + t, vh_item(1, t, xv1_of)))
            laneA.extend(sorted(items, key=lambda x: x[0]))

            def gate_b0sc0(t):
                return 19 + t

            attention(0, qh0, kh0, lambda: hold["vh0"], gate0=gate_b0sc0)
            attention(1, hold["qh"], hold["kh"], lambda: hold["vh1"],
                      last=True)

            while laneA or laneB or laneC or laneD:
                progressed = False
                if laneA:
                    laneA.pop(0)[1]()
                    progressed = True
                if laneB and (laneB[0][1]() or not progressed):
                    laneB.pop(0)[2]()
                    progressed = True
                if laneC and (laneC[0][1]() or not progressed):
                    laneC.pop(0)[2]()
                    progressed = True
                if laneD and (laneD[0][1]() or not progressed):
                    laneD.pop(0)[2]()

    nc.compile()
    return nc


def make_in_maps(q, k, v, Wq, bq, Wo):
    bf = ml_dtypes.bfloat16
    xT = {}
    for name, x in (("qT", q), ("kT", k), ("vT", v)):
        # per-(batch, j) blocks [128, ND*512]: block (b, j) holds
        # x[b, j*512:(j+1)*512, :] with layout [p, d*512 + c] =
        # x[b, j*512 + c, d*128 + p] -> 8KB contiguous per partition
        xa = np.asarray(x, np.float32).reshape(B, NSC, 512, ND, 128)
        xT[name] = np.ascontiguousarray(
            xa.transpose(4, 0, 1, 3, 2).reshape(128, B * NSC * BLK)
        ).astype(bf)

    in_maps = []
    for c in range(NCORES):
        cols = slice(c * HD, (c + 1) * HD)
        wqc = np.asarray(Wq, np.float32)[:, cols]
        bqc = np.asarray(bq, np.float32)[cols]
        wqve = np.zeros((D, 130), np.float32)
        wqve[:, 0:64] = wqc[:, 0:64]
        wqve[:, 65:129] = wqc[:, 64:128]
        bqve = np.zeros((1, 130), np.float32)
        bqve[0, 0:64] = bqc[0:64]
        bqve[0, 65:129] = bqc[64:128]
        bqve[0, 64] = 1.0
        bqve[0, 129] = 1.0
        # d-major packing: wq_pk[p, d*HD+c] = wqc[d*128+p, c]
        wq_pk = np.ascontiguousarray(
            wqc.reshape(ND, 128, HD).transpose(1, 0, 2).reshape(128, ND * HD))
        wqv_pk = np.ascontiguousarray(
            wqve.reshape(ND, 128, 130).transpose(1, 0, 2).reshape(128, ND * 130))
        sel2 = np.zeros((2, 128), np.float32)
        sel2[0, 0:64] = 1.0
        sel2[1, 64:128] = 1.0
        in_maps.append({
            "qT": xT["qT"], "kT": xT["kT"], "vT": xT["vT"],
            "wq": wq_pk.astype(bf),
            "wqv": wqv_pk.astype(bf),
            "bqc": np.ascontiguousarray(bqc[:, None]),
            "bqvb": np.ascontiguousarray(np.tile(bqve, (128, 1))),
            "wo": np.ascontiguousarray(np.asarray(Wo, np.float32)[cols, :]).astype(bf),
            "sel2": sel2.astype(bf),
        })
    return in_maps


def kernel(q, k, v, Wq, bq, Wo, bo):
    import jax
    from concourse.bass_utils import run_bass_kernel_spmd

    try:
        jax.config.update("jax_compilation_cache_dir", "/tmp/jax_bass_cache")
        jax.config.update("jax_persistent_cache_min_entry_size_bytes", -1)
        jax.config.update("jax_persistent_cache_min_compile_time_secs", 0)
    except Exception:
        pass

    if "nc" not in _cache:
        _cache["nc"] = _build()
    nc = _cache["nc"]

    in_maps = make_in_maps(q, k, v, Wq, bq, Wo)
    res = run_bass_kernel_spmd(nc, in_maps, list(range(NCORES)), trace=False)
    acc = np.zeros((BS, D), np.float64)
    for c in range(NCORES):
        acc += res.results[c]["out"].astype(np.float64)
    acc += np.asarray(bo, np.float32)[None, :].astype(np.float64)
    return acc.reshape(B, S, D).astype(np.float32)


# revision 57
# speedup vs baseline: 1.0130x; 1.0130x over previous
"""Multi-head attention (B=2, S=2048, D=1024, H=16, d_k=64) on 8 TRN2 NeuronCores.

Sharding: head-parallel. Core c owns heads (2c, 2c+1) for both batch rows:
 - replicated inputs: qT/kT/vT host-packed as per-(batch, 512-col j-block)
   blocks [128, ND*512] so each block is ONE [128, 4096] DMA with an 8KB
   contiguous run per partition (full hardware-DGE rate, cheap trigger);
   within a block the 8 d-chunks are column-major groups, d on partitions
   so the TensorEngine contracts over D with no transposes.
 - per-core weights: Wq columns / Wo rows for its two heads (host pre-packs
   wq/wqv d-major so each is ONE contiguous DMA)
 - per-core output: partial = attn_out(own heads) @ Wo[own rows]  [4096, 1024] bf16
   The host sums the 8 partials (f32) and adds bo.  No cross-core comm.

Per-core dataflow (bf16 matmuls, f32 PSUM):
 1. wq + bq + all x blocks ride the sync hardware-DMA queue (the gpsimd
    software queue starts ~10us late and runs ~3x slower -- only the
    late-needed consts and b0 stores go there).  Sync order: q(b0,j0),
    k(b0,j0..3), q(b0,j1..3), v(b0,j0..3), then b1 q/k/v via lane-A
    thunks.  qh-j0 projects as soon as its block lands; kh j0..3
    projections chase their DMAs as lane-A items inside the t-loop, so
    scores(sc0) start ~7us after the first matmul instead of waiting for
    all of k.
 2. qhT/khT [128, 2048] per batch = Wq_c.T @ xT (+bq).  vh [2048, 130]
    natural = vT.T @ Wqv_c; Wqv has zero-cols / bqv has 1.0-cols so each
    head gets a ones column -> attn@V also produces softmax denominators.
 3. scoresT[t,s] = khT.T @ qhT, both heads packed into disjoint PE
    row-groups (K=64).  exp(x/8) on ScalarE from PSUM, bf16 out.
 4. attn@V accumulated over t; row 64 = denominator.  Normalize: DVE
    fast-reciprocal straight off the two denominator rows into a [1,2,512]
    f32 tile, bf16 rounding copy, per-head K=1 matmuls against the sel2
    ones row broadcast the reciprocals across partitions, then DVE muls
    (att sbuf x bcast psum) produce normalized oT bf16.
 5. partial[s, :] = outT.T @ Wo_c -> bf16 ob [128,1024] (two PSUM copies)
    -> ONE DRAM store per 128 rows.  b0 stores ride gpsimd; b1 stores ride
    sync (free after the b1 loads), so the tail drain is short.

Scheduling: ScalarE (exp, ~143us) and the TensorEngine (~170us execute) --
PE is the bottleneck, so emission keeps the PE queue dense: each s-chunk's
scores+exp loop is emitted first; its attn@V/normalize/out-proj are
deferred one s-chunk and re-emitted between later score iterations via a
three-lane work queue (lane A: DMA-gated projection work with
earliest-iteration thresholds; lane B: deferred attention work, also
min-iter gated so no PE instruction is emitted before its input DMA can
have landed -- the PE queue is in-order, a stalled instruction blocks it).
"""

import numpy as np
import ml_dtypes

B, S, D, H, DK = 2, 2048, 1024, 16, 64
NCORES = 8
HPC = H // NCORES          # heads per core = 2
BS = B * S                 # 4096
HD = HPC * DK              # 128 = per-core head dims
ND = D // 128              # 8 d-chunks
NSC = S // 512             # 4 column blocks per batch
BLK = ND * 512             # 4096 elems per partition per block

_cache = {}


def _build():
    import concourse.bass as bass
    import concourse.tile as tile
    from concourse import bacc, mybir

    f32 = mybir.dt.float32
    bf16 = mybir.dt.bfloat16
    Exp = mybir.ActivationFunctionType.Exp

    nc = bacc.Bacc("TRN2", target_bir_lowering=False, debug=False,
                   num_devices=NCORES)

    qT = nc.declare_dram_parameter("qT", [128, B * NSC * BLK], bf16,
                                   isOutput=False)
    kT = nc.declare_dram_parameter("kT", [128, B * NSC * BLK], bf16,
                                   isOutput=False)
    vT = nc.declare_dram_parameter("vT", [128, B * NSC * BLK], bf16,
                                   isOutput=False)
    wq = nc.declare_dram_parameter("wq", [128, ND * HD], bf16, isOutput=False)
    wqv = nc.declare_dram_parameter("wqv", [128, ND * 130], bf16, isOutput=False)
    bqc = nc.declare_dram_parameter("bqc", [HD, 1], f32, isOutput=False)
    bqvb = nc.declare_dram_parameter("bqvb", [128, 130], f32, isOutput=False)
    wo = nc.declare_dram_parameter("wo", [HD, D], bf16, isOutput=False)
    sel2d = nc.declare_dram_parameter("sel2", [2, 128], bf16, isOutput=False)
    out = nc.declare_dram_parameter("out", [BS, D], bf16, isOutput=True)

    NT = S // 128            # 16 t-chunks per batch

    with tile.TileContext(nc) as tc:
        with (
            tc.tile_pool(name="const", bufs=1) as pc,
            tc.tile_pool(name="xg", bufs=12) as pin1,
            tc.tile_pool(name="proj", bufs=2) as pproj,
            tc.tile_pool(name="vh", bufs=2) as pvh,
            tc.tile_pool(name="exp", bufs=19) as pexp,
            tc.tile_pool(name="outT", bufs=2) as poutT,
            tc.tile_pool(name="small", bufs=2) as psmall,
            tc.tile_pool(name="ob", bufs=3) as pob,
            tc.tile_pool(name="ps", bufs=2, space="PSUM") as pps,
        ):
            # ---- sync hardware queue: wq + bq first (first proj needs
            # them), then q j0, k j0..3, q j1..3, v j0..3
            wq_sb = pc.tile([128, ND * HD], bf16)
            nc.sync.dma_start(wq_sb[:], wq[:, :])
            bq_col = pc.tile([128, 1], f32)
            nc.sync.dma_start(bq_col[:], bqc[:, :])

            def dma_blk(src, b, j, nm):
                t = pin1.tile([128, BLK], bf16, tag="xg",
                              name=f"x{nm}{b}{j}")
                off = (b * NSC + j) * BLK
                nc.sync.dma_start(t[:], src[:, off:off + BLK])
                return t

            qx0 = [None] * NSC
            kx0 = [None] * NSC
            vx0 = [None] * NSC
            qx0[0] = dma_blk(qT, 0, 0, "q")
            for j in range(NSC):
                kx0[j] = dma_blk(kT, 0, j, "k")
            for j in range(1, NSC):
                qx0[j] = dma_blk(qT, 0, j, "q")
            for j in range(NSC):
                vx0[j] = dma_blk(vT, 0, j, "v")

            # ---- gpsimd software queue: only late-needed consts
            bqv_bc = pc.tile([128, 130], f32)
            nc.gpsimd.dma_start(bqv_bc[:], bqvb[:, :])
            wqv_sb = pc.tile([128, ND * 130], bf16)
            nc.gpsimd.dma_start(wqv_sb[:], wqv[:, :])
            wo_sb = pc.tile([HD, D], bf16)
            nc.gpsimd.dma_start(wo_sb[:], wo[:, :])
            # [2,128] bf16 selector (host-packed): row 0 is ones on cols
            # 0:64 (the K=1 broadcast stationary); row h is 1 on head h's
            # 64 cols
            sel2 = pc.tile([2, 128], bf16)
            nc.gpsimd.dma_start(sel2[:], sel2d[:, :])


            def wqd(d):
                return wq_sb[:, d * HD:(d + 1) * HD]

            def wqvd(d):
                return wqv_sb[:, d * 130:(d + 1) * 130]

            qh0 = pproj.tile([128, S], bf16, tag="projq", name="projq0")
            kh0 = pproj.tile([128, S], bf16, tag="projk", name="projk0")

            # one j-block projection: 8 accumulating matmuls + bias add
            def proj_j(xt, sb, j):
                ps = pps.tile([128, 512], f32, tag="p1", name=f"pj{j}")
                for d in range(ND):
                    nc.tensor.matmul(ps, wqd(d),
                                     xt[:, d * 512:(d + 1) * 512],
                                     start=(d == 0), stop=(d == ND - 1))
                nc.vector.tensor_scalar_add(
                    sb[:, j * 512:(j + 1) * 512], ps, bq_col[:])

            # qh j0 + kh j0 before the t-loop; kh j1..3 chase their DMAs
            # as lane-A items inside it.
            proj_j(qx0[0], qh0, 0)
            proj_j(kx0[0], kh0, 0)

            # ---- three-lane deferred work queue ----
            laneA = []   # (min_iter, thunk): DMA-gated projection work
            laneB = []   # (min_iter, ready_fn, thunk): attn@V + asb copies
            laneC = []   # (min_iter, ready_fn, thunk): norm + out-proj
            it = [0]
            done = {}    # emission flags: (name, t) -> True

            def pump():
                popped = 0
                if laneA and laneA[0][0] <= it[0]:
                    laneA.pop(0)[1]()
                    popped = 1
                for _ in range(2 - popped):
                    if laneB and laneB[0][0] <= it[0] and laneB[0][1]():
                        laneB.pop(0)[2]()
                if laneC and laneC[0][0] <= it[0] and laneC[0][1]():
                    laneC.pop(0)[2]()
                it[0] += 1

            hold = {}

            # vh items: [128, 130] t-chunks; block j = t//4
            def vh_item(b, t, xv_of):
                def tt():
                    ps = pps.tile([128, 130], f32, tag="p1", name=f"pvh{b}{t}")
                    for d in range(ND):
                        nc.tensor.matmul(ps, xv_of(d, t), wqvd(d),
                                         start=(d == 0), stop=(d == ND - 1))
                    nc.vector.tensor_add(hold["vh" + str(b)][:, t, :],
                                         ps[:], bqv_bc[:])
                    done[("vh" + str(b), t)] = True
                return tt

            def xv_slice(blocks, d, t):
                c = d * 512 + (t % 4) * 128
                return blocks[t // 4][:, c:c + 128]

            def xv0_of(d, t):
                return xv_slice(vx0, d, t)

            def xv1_of(d, t):
                return xv_slice(hold["vt1"], d, t)

            # b1 q/k block loads + projections as lane-A items.  Each b1
            # block DMA reuses an xg buf whose b0 reader must already be
            # EMITTED (Tile WAR deps only see emitted readers), so each
            # load is its own item gated just past that reader's item.
            def qk_chain_thunks(base, step):
                items = []

                def alloc_thunk():
                    hold["qt1"] = [None] * NSC
                    hold["kt1"] = [None] * NSC
                    hold["qh"] = pproj.tile([128, S], bf16, tag="projq",
                                            name="projq1")
                    hold["kh"] = pproj.tile([128, S], bf16, tag="projk",
                                            name="projk1")
                    hold["qt1"][0] = dma_blk(qT, 1, 0, "q")
                    hold["qt1"][1] = dma_blk(qT, 1, 1, "q")
                items.append((1, alloc_thunk))

                def b1_dma(tgt, src, j, nm):
                    def th():
                        hold[tgt][j] = dma_blk(src, 1, j, nm)
                    return th
                items.append((6, b1_dma("qt1", qT, 2, "q")))
                items.append((9, b1_dma("qt1", qT, 3, "q")))
                items.append((11, b1_dma("kt1", kT, 0, "k")))
                items.append((13, b1_dma("kt1", kT, 1, "k")))
                items.append((16, b1_dma("kt1", kT, 2, "k")))
                items.append((18, b1_dma("kt1", kT, 3, "k")))
                cell = {}
                for i, name in enumerate(("q", "k")):
                    for j in range(NSC):
                        def t1a(name=name, j=j):
                            ps = pps.tile([128, 512], f32,
                                          tag="p1", name=f"pb{name}{j}")
                            xt = hold["qt1" if name == "q" else "kt1"][j]
                            for d in range(4):
                                nc.tensor.matmul(
                                    ps, wqd(d), xt[:, d * 512:(d + 1) * 512],
                                    start=(d == 0), stop=False)
                            cell[(name, j)] = ps

                        def t1b(name=name, j=j):
                            ps = cell[(name, j)]
                            xt = hold["qt1" if name == "q" else "kt1"][j]
                            for d in range(4, ND):
                                nc.tensor.matmul(
                                    ps, wqd(d), xt[:, d * 512:(d + 1) * 512],
                                    start=False,
                                    stop=(d == ND - 1))
                            sb = hold["qh" if name == "q" else "kh"]
                            nc.vector.tensor_scalar_add(
                                sb[:, j * 512:(j + 1) * 512], ps, bq_col[:])
                        items.append((base[i] + step * j, t1a))
                        items.append((base[i] + step * j, t1b))
                return items

            f32r = mybir.dt.float32r

            def norm_rec_a(att, sfx, den_eng=None):
                # den rows copied to partition 0 first: the custom-DVE
                # reciprocal mislowers partition-offset inputs on HW.
                den = psmall.tile([1, 2, 512], f32, tag="dcp", bufs=1,
                                  name="den" + sfx)
                nc.vector.tensor_copy(den[0:1, 0, :], att[0][64:65, :])
                if den_eng is None:
                    nc.vector.tensor_copy(den[0:1, 1, :], att[1][64:65, :])
                else:
                    den_eng.copy(den[0:1, 1, :], att[1][64:65, :])
                rec = psmall.tile([1, 2, 512], f32, tag="den", bufs=1,
                                  name="rec" + sfx)
                nc.vector.reciprocal_approx_fast(rec[:], den[:])
                recb = psmall.tile([1, 2, 512], bf16, tag="recb", bufs=1,
                                   name="recb" + sfx)
                nc.vector.tensor_copy(recb[:], rec[:])
                return recb

            def norm_rec_b(recb, sfx):
                # per-head K=1 broadcast matmuls against the sel2 ones row
                bcds = []
                for h in range(HPC):
                    bcd = pps.tile([64, 512], f32, tag="p1",
                                   name=f"bcd{h}" + sfx)
                    nc.tensor.matmul(bcd, sel2[0:1, 0:64], recb[0:1, h, :],
                                     start=True, stop=True)
                    bcds.append(bcd)
                return bcds

            def norm_v2(att, oT, ssl, sfx):
                # deferred path: att is sbuf (asb); muls read bcd psum
                bcds = norm_rec_b(norm_rec_a(att, sfx), sfx)
                for h in range(HPC):
                    hp = slice(h * 64, (h + 1) * 64)
                    nc.vector.tensor_mul(oT[hp, ssl],
                                         att[h][0:64, :], bcds[h][:, :])

            def outproj_s1(b, sc, s1, oT, sfx, cast_eng=None,
                           split_store=False):
                s0 = sc * 512 + s1 * 128
                rs = slice(b * S + s0, b * S + s0 + 128)
                ob = pob.tile([128, D], bf16, tag="ob", name="ob" + sfx)
                # b0 stores ride the gpsimd software queue; b1 stores ride
                # sync (free after the b1 loads) so the tail drains fast
                seng = nc.gpsimd if b == 0 else nc.sync
                for n in range(2):
                    nsl = slice(n * 512, (n + 1) * 512)
                    ps = pps.tile([128, 512], f32, tag="p1",
                                  name="opps" + sfx)
                    nc.tensor.matmul(ps, oT[:, s0:s0 + 128], wo_sb[:, nsl],
                                     start=True, stop=True)
                    if cast_eng is None:
                        nc.vector.tensor_copy(ob[:, nsl], ps)
                    else:
                        cast_eng.copy(ob[:, nsl], ps)
                    if split_store:
                        eng = nc.gpsimd if n == 0 else nc.sync
                        eng.dma_start(out[rs, nsl], ob[:, nsl])
                if not split_store:
                    seng.dma_start(out[rs, :], ob[:])

            def defer_attnv(b, sc, exs, vh_of, oT, gate, og):
                ssl = slice(sc * 512, (sc + 1) * 512)
                cell = {}
                for t in range(NT):
                    def av(t=t):
                        if t == 0:
                            cell["att"] = [
                                pps.tile([65, 512], f32, tag="att",
                                         name=f"att{b}{sc}{h}")
                                for h in range(HPC)]
                        vh = vh_of()
                        for h in range(HPC):
                            nc.tensor.matmul(cell["att"][h],
                                             vh[:, t, h * 65:h * 65 + 65],
                                             exs[t][:, h, :],
                                             start=(t == 0), stop=(t == NT - 1))
                    g = gate(t) if gate is not None else 0
                    laneB.append((g,
                                  (lambda t=t: ("vh" + str(b), t) in done),
                                  av))

                # asb copies (DVE-only) free the att psum bank promptly so
                # the next s-chunk's attn@V can start; the norm matmul +
                # out-proj go to laneC, gated past the last attn@V so score
                # matmuls sit between them in the in-order PE queue, hiding
                # the DVE reciprocal-chain latency.
                def asb_copy():
                    cell["asb"] = [
                        psmall.tile([65, 512], f32, tag="asb", bufs=4,
                                    name=f"asb{b}{sc}{h}")
                        for h in range(HPC)]
                    for h in range(HPC):
                        nc.vector.tensor_copy(cell["asb"][h][:],
                                              cell["att"][h][:])
                laneB.append((0, lambda: True, asb_copy))
                d0 = it[0]

                def norm():
                    norm_v2(cell["asb"], oT, ssl, f"{b}{sc}")
                    cell["normed"] = True
                laneC.append((d0 + 13, lambda: "asb" in cell, norm))

                for g in range(2):
                    def op(g=g):
                        for u in range(2):
                            outproj_s1(b, sc, g * 2 + u, oT, f"{b}{sc}")
                    laneC.append((og[g], lambda: "normed" in cell, op))

            def inline_tail(b, sc, att, oT):
                # att is psum here.  den-h1 copy rides ScalarE (exps are
                # done; the COPY table is shared with the ob casts), bcs
                # copies keep the muls off dual-psum reads, and each
                # u-chunk's muls feed its outproj immediately.  The last
                # deferred outproj group is drained between the norm
                # phases: real PE work covering the DVE reciprocal chain.
                ssl = slice(sc * 512, (sc + 1) * 512)
                recb = norm_rec_a(att, "L", den_eng=nc.scalar)
                bcds = norm_rec_b(recb, "L")
                bcss = []
                for h in range(HPC):
                    bcs = psmall.tile([64, 512], f32, tag="bcs", bufs=1,
                                      name=f"bcs{h}L")
                    nc.vector.tensor_copy(bcs[:], bcds[h][:, :])
                    bcss.append(bcs)
                for u in range(4):
                    us = slice(u * 128, (u + 1) * 128)
                    for h in range(HPC):
                        hp = slice(h * 64, (h + 1) * 64)
                        nc.vector.tensor_mul(oT[hp, ssl][:, us],
                                             att[h][0:64, us], bcss[h][:, us])
                    eng = nc.scalar if u % 2 else None
                    outproj_s1(b, sc, u, oT, "L", cast_eng=eng,
                               split_store=(u == 3))

            def attention(b, qh, kh, vh_of, gate0=None, last=False):
                oT = poutT.tile([128, S], bf16, tag="outT", name=f"oT{b}")
                for sc in range(NSC):
                    inline = last and sc == NSC - 1
                    exs = []
                    att = None
                    for t in range(NT):
                        pump()
                        scps = pps.tile([128, HPC, 512], f32, tag="sc")
                        for h in range(HPC):
                            hp = slice(h * 64, (h + 1) * 64)
                            nc.tensor.matmul(scps[:, h, :],
                                             kh[hp, t * 128:(t + 1) * 128],
                                             qh[hp, sc * 512:(sc + 1) * 512],
                                             start=True, stop=True)
                        ex = pexp.tile([128, HPC, 512], bf16, tag="exp")
                        nc.scalar.activation(ex[:], scps[:], Exp, scale=0.125)
                        exs.append(ex)
                        if inline:
                            if att is None:
                                att = [pps.tile([65, 512], f32, tag="att",
                                                name=f"attL{h}")
                                       for h in range(HPC)]
                            vh = vh_of()
                            for h in range(HPC):
                                nc.tensor.matmul(att[h],
                                                 vh[:, t, h * 65:h * 65 + 65],
                                                 ex[:, h, :],
                                                 start=(t == 0),
                                                 stop=(t == NT - 1))
                    if inline:
                        inline_tail(b, sc, att, oT)
                    else:
                        gate = gate0 if (gate0 is not None and sc == 0) \
                            else None
                        # outproj spread through the next window; b1-sc2's
                        # goes to the inline tail's manual drain (129/130
                        # never pop in-loop)
                        og = (it[0] + 14, it[0] + 15)
                        defer_attnv(b, sc, exs, vh_of, oT, gate, og)

            # ---- lane-A schedule ----
            # iters are scores-iterations (~1.1us each from ~13us).
            # landings (sync hw queue, ramping ~170->280GB/s): k j1/2/3
            # ~18/21/23us -> kh-j at 5/8/10; q j1..3 ~26/29/31 -> qh-j at
            # 12/15/17; v0 j-blocks ~34/37/40/43 -> vh t at 19+t; b1 q1
            # ~46-55 -> 30+3j, k1 ~58-67 -> 41+3j, v1 ~70-79 -> vh1 52+t.
            hold["vh0"] = pvh.tile([128, NT, 130], bf16, tag="vh", name="vh0")
            items = [(4, lambda: proj_j(kx0[1], kh0, 1)),
                     (8, lambda: proj_j(kx0[2], kh0, 2)),
                     (10, lambda: proj_j(kx0[3], kh0, 3)),
                     (12, lambda: proj_j(qx0[1], qh0, 1)),
                     (15, lambda: proj_j(qx0[2], qh0, 2)),
                     (17, lambda: proj_j(qx0[3], qh0, 3))]
            for t in range(NT):
                items.append((19 + t, vh_item(0, t, xv0_of)))
            items += qk_chain_thunks((30, 41), 3)

            def v1_first():
                hold["vt1"] = [None] * NSC
                hold["vh1"] = pvh.tile([128, NT, 130], bf16, tag="vh",
                                       name="vh1")
                hold["vt1"][0] = dma_blk(vT, 1, 0, "v")
            items.append((23, v1_first))
            for j in range(1, NSC):
                def v1_blk(j=j):
                    hold["vt1"][j] = dma_blk(vT, 1, j, "v")
                items.append((23 + 4 * j, v1_blk))
            for t in range(NT):
                items.append((52 + t, vh_item(1, t, xv1_of)))
            laneA.extend(sorted(items, key=lambda x: x[0]))

            def gate_b0sc0(t):
                return 19 + t

            attention(0, qh0, kh0, lambda: hold["vh0"], gate0=gate_b0sc0)
            attention(1, hold["qh"], hold["kh"], lambda: hold["vh1"],
                      last=True)

            while laneA or laneB or laneC:
                progressed = False
                if laneA:
                    laneA.pop(0)[1]()
                    progressed = True
                if laneB and (laneB[0][1]() or not progressed):
                    laneB.pop(0)[2]()
                    progressed = True
                if laneC and (laneC[0][1]() or not progressed):
                    laneC.pop(0)[2]()

    nc.compile()
    return nc


def make_in_maps(q, k, v, Wq, bq, Wo):
    bf = ml_dtypes.bfloat16
    xT = {}
    for name, x in (("qT", q), ("kT", k), ("vT", v)):
        # per-(batch, j) blocks [128, ND*512]: block (b, j) holds
        # x[b, j*512:(j+1)*512, :] with layout [p, d*512 + c] =
        # x[b, j*512 + c, d*128 + p] -> 8KB contiguous per partition
        xa = np.asarray(x, np.float32).reshape(B, NSC, 512, ND, 128)
        xT[name] = np.ascontiguousarray(
            xa.transpose(4, 0, 1, 3, 2).reshape(128, B * NSC * BLK)
        ).astype(bf)

    in_maps = []
    for c in range(NCORES):
        cols = slice(c * HD, (c + 1) * HD)
        wqc = np.asarray(Wq, np.float32)[:, cols]
        bqc = np.asarray(bq, np.float32)[cols]
        wqve = np.zeros((D, 130), np.float32)
        wqve[:, 0:64] = wqc[:, 0:64]
        wqve[:, 65:129] = wqc[:, 64:128]
        bqve = np.zeros((1, 130), np.float32)
        bqve[0, 0:64] = bqc[0:64]
        bqve[0, 65:129] = bqc[64:128]
        bqve[0, 64] = 1.0
        bqve[0, 129] = 1.0
        # d-major packing: wq_pk[p, d*HD+c] = wqc[d*128+p, c]
        wq_pk = np.ascontiguousarray(
            wqc.reshape(ND, 128, HD).transpose(1, 0, 2).reshape(128, ND * HD))
        wqv_pk = np.ascontiguousarray(
            wqve.reshape(ND, 128, 130).transpose(1, 0, 2).reshape(128, ND * 130))
        sel2 = np.zeros((2, 128), np.float32)
        sel2[0, 0:64] = 1.0
        sel2[1, 64:128] = 1.0
        in_maps.append({
            "qT": xT["qT"], "kT": xT["kT"], "vT": xT["vT"],
            "wq": wq_pk.astype(bf),
            "wqv": wqv_pk.astype(bf),
            "bqc": np.ascontiguousarray(bqc[:, None]),
            "bqvb": np.ascontiguousarray(np.tile(bqve, (128, 1))),
            "wo": np.ascontiguousarray(np.asarray(Wo, np.float32)[cols, :]).astype(bf),
            "sel2": sel2.astype(bf),
        })
    return in_maps


def kernel(q, k, v, Wq, bq, Wo, bo):
    import jax
    from concourse.bass_utils import run_bass_kernel_spmd

    try:
        jax.config.update("jax_compilation_cache_dir", "/tmp/jax_bass_cache")
        jax.config.update("jax_persistent_cache_min_entry_size_bytes", -1)
        jax.config.update("jax_persistent_cache_min_compile_time_secs", 0)
    except Exception:
        pass

    if "nc" not in _cache:
        _cache["nc"] = _build()
    nc = _cache["nc"]

    in_maps = make_in_maps(q, k, v, Wq, bq, Wo)
    res = run_bass_kernel_spmd(nc, in_maps, list(range(NCORES)), trace=False)
    acc = np.zeros((BS, D), np.float64)
    for c in range(NCORES):
        acc += res.results[c]["out"].astype(np.float64)
    acc += np.asarray(bo, np.float32)[None, :].astype(np.float64)
    return acc.reshape(B, S, D).astype(np.float32)


# revision 60
# speedup vs baseline: 1.0364x; 1.0231x over previous
"""Multi-head attention (B=2, S=2048, D=1024, H=16, d_k=64) on 8 TRN2 NeuronCores.

Sharding: head-parallel. Core c owns heads (2c, 2c+1) for both batch rows:
 - replicated inputs: qT/kT/vT host-packed as per-(batch, 512-col j-block)
   blocks [128, ND*512] so each block is ONE [128, 4096] DMA with an 8KB
   contiguous run per partition (full hardware-DGE rate, cheap trigger);
   within a block the 8 d-chunks are column-major groups, d on partitions
   so the TensorEngine contracts over D with no transposes.
 - per-core weights: Wq columns / Wo rows for its two heads (host pre-packs
   wq/wqv d-major so each is ONE contiguous DMA)
 - per-core output: partial = attn_out(own heads) @ Wo[own rows]  [4096, 1024] bf16
   The host sums the 8 partials (f32) and adds bo.  No cross-core comm.

Per-core dataflow (bf16 matmuls, f32 PSUM):
 1. wq + bq + all x blocks ride the sync hardware-DMA queue (the gpsimd
    software queue starts ~10us late and runs ~3x slower -- only the
    late-needed consts and b0 stores go there).  Sync order: q(b0,j0),
    k(b0,j0..3), q(b0,j1..3), v(b0,j0..3), then b1 q/k/v via lane-A
    thunks.  qh-j0 projects as soon as its block lands; kh j0..3
    projections chase their DMAs as lane-A items inside the t-loop, so
    scores(sc0) start ~7us after the first matmul instead of waiting for
    all of k.
 2. qhT/khT [128, 2048] per batch = Wq_c.T @ xT (+bq).  vh [2048, 130]
    natural = vT.T @ Wqv_c; Wqv has zero-cols / bqv has 1.0-cols so each
    head gets a ones column -> attn@V also produces softmax denominators.
 3. scoresT[t,s] = khT.T @ qhT, both heads packed into disjoint PE
    row-groups (K=64).  exp(x/8) on ScalarE from PSUM, bf16 out.
 4. attn@V accumulated over t; row 64 = denominator.  Normalize: DVE
    fast-reciprocal straight off the two denominator rows into a [1,2,512]
    f32 tile, bf16 rounding copy, per-head K=1 matmuls against the sel2
    ones row broadcast the reciprocals across partitions, then DVE muls
    (att sbuf x bcast psum) produce normalized oT bf16.
 5. partial[s, :] = outT.T @ Wo_c -> bf16 ob [128,1024] (two PSUM copies)
    -> ONE DRAM store per 128 rows.  b0 stores ride gpsimd; b1 stores ride
    sync (free after the b1 loads), so the tail drain is short.

Scheduling: ScalarE (exp, ~143us) and the TensorEngine (~170us execute) --
PE is the bottleneck, so emission keeps the PE queue dense: each s-chunk's
scores+exp loop is emitted first; its attn@V/normalize/out-proj are
deferred one s-chunk and re-emitted between later score iterations via a
three-lane work queue (lane A: DMA-gated projection work with
earliest-iteration thresholds; lane B: deferred attention work, also
min-iter gated so no PE instruction is emitted before its input DMA can
have landed -- the PE queue is in-order, a stalled instruction blocks it).
"""

import numpy as np
import ml_dtypes

B, S, D, H, DK = 2, 2048, 1024, 16, 64
NCORES = 8
HPC = H // NCORES          # heads per core = 2
BS = B * S                 # 4096
HD = HPC * DK              # 128 = per-core head dims
ND = D // 128              # 8 d-chunks
NSC = S // 512             # 4 column blocks per batch
BLK = ND * 512             # 4096 elems per partition per block

_cache = {}


def _build():
    import concourse.bass as bass
    import concourse.tile as tile
    from concourse import bacc, mybir

    f32 = mybir.dt.float32
    bf16 = mybir.dt.bfloat16
    Exp = mybir.ActivationFunctionType.Exp

    nc = bacc.Bacc("TRN2", target_bir_lowering=False, debug=False,
                   num_devices=NCORES)

    qT = nc.declare_dram_parameter("qT", [128, B * NSC * BLK], bf16,
                                   isOutput=False)
    kT = nc.declare_dram_parameter("kT", [128, B * NSC * BLK], bf16,
                                   isOutput=False)
    vT = nc.declare_dram_parameter("vT", [128, B * NSC * BLK], bf16,
                                   isOutput=False)
    wq = nc.declare_dram_parameter("wq", [128, ND * HD], bf16, isOutput=False)
    wqv = nc.declare_dram_parameter("wqv", [128, ND * 130], bf16, isOutput=False)
    bqc = nc.declare_dram_parameter("bqc", [HD, 1], f32, isOutput=False)
    bqvb = nc.declare_dram_parameter("bqvb", [128, 130], f32, isOutput=False)
    wo = nc.declare_dram_parameter("wo", [HD, D], bf16, isOutput=False)
    sel2d = nc.declare_dram_parameter("sel2", [2, 128], bf16, isOutput=False)
    out = nc.declare_dram_parameter("out", [BS, D], bf16, isOutput=True)

    NT = S // 128            # 16 t-chunks per batch

    with tile.TileContext(nc) as tc:
        with (
            tc.tile_pool(name="const", bufs=1) as pc,
            tc.tile_pool(name="xg", bufs=12) as pin1,
            tc.tile_pool(name="proj", bufs=2) as pproj,
            tc.tile_pool(name="vh", bufs=2) as pvh,
            tc.tile_pool(name="exp", bufs=19) as pexp,
            tc.tile_pool(name="outT", bufs=2) as poutT,
            tc.tile_pool(name="small", bufs=2) as psmall,
            tc.tile_pool(name="ob", bufs=3) as pob,
            tc.tile_pool(name="ps", bufs=2, space="PSUM") as pps,
        ):
            # ---- sync hardware queue: wq + bq first (first proj needs
            # them), then q j0, k j0..3, q j1..3, v j0..3
            wq_sb = pc.tile([128, ND * HD], bf16)
            nc.sync.dma_start(wq_sb[:], wq[:, :])
            bq_col = pc.tile([128, 1], f32)
            nc.sync.dma_start(bq_col[:], bqc[:, :])

            def dma_blk(src, b, j, nm):
                t = pin1.tile([128, BLK], bf16, tag="xg",
                              name=f"x{nm}{b}{j}")
                off = (b * NSC + j) * BLK
                nc.sync.dma_start(t[:], src[:, off:off + BLK])
                return t

            qx0 = [None] * NSC
            kx0 = [None] * NSC
            vx0 = [None] * NSC
            qx0[0] = dma_blk(qT, 0, 0, "q")
            for j in range(NSC):
                kx0[j] = dma_blk(kT, 0, j, "k")
            for j in range(1, NSC):
                qx0[j] = dma_blk(qT, 0, j, "q")
            for j in range(NSC):
                vx0[j] = dma_blk(vT, 0, j, "v")

            # ---- gpsimd software queue: only late-needed consts
            bqv_bc = pc.tile([128, 130], f32)
            nc.gpsimd.dma_start(bqv_bc[:], bqvb[:, :])
            wqv_sb = pc.tile([128, ND * 130], bf16)
            nc.gpsimd.dma_start(wqv_sb[:], wqv[:, :])
            wo_sb = pc.tile([HD, D], bf16)
            nc.gpsimd.dma_start(wo_sb[:], wo[:, :])
            # [2,128] bf16 selector (host-packed): row 0 is ones on cols
            # 0:64 (the K=1 broadcast stationary); row h is 1 on head h's
            # 64 cols
            sel2 = pc.tile([2, 128], bf16)
            nc.gpsimd.dma_start(sel2[:], sel2d[:, :])


            def wqd(d):
                return wq_sb[:, d * HD:(d + 1) * HD]

            def wqvd(d):
                return wqv_sb[:, d * 130:(d + 1) * 130]

            qh0 = pproj.tile([128, S], bf16, tag="projq", name="projq0")
            kh0 = pproj.tile([128, S], bf16, tag="projk", name="projk0")

            # one j-block projection: 8 accumulating matmuls + bias add
            def proj_j(xt, sb, j):
                ps = pps.tile([128, 512], f32, tag="p1", name=f"pj{j}")
                for d in range(ND):
                    nc.tensor.matmul(ps, wqd(d),
                                     xt[:, d * 512:(d + 1) * 512],
                                     start=(d == 0), stop=(d == ND - 1))
                nc.vector.tensor_scalar_add(
                    sb[:, j * 512:(j + 1) * 512], ps, bq_col[:])

            # qh j0 + kh j0 before the t-loop; kh j1..3 chase their DMAs
            # as lane-A items inside it.
            proj_j(qx0[0], qh0, 0)
            proj_j(kx0[0], kh0, 0)

            # ---- three-lane deferred work queue ----
            laneA = []   # (min_iter, thunk): DMA-gated projection work
            laneB = []   # (min_iter, ready_fn, thunk): attn@V + asb copies
            laneC = []   # (min_iter, ready_fn, thunk): norm + out-proj
            it = [0]
            done = {}    # emission flags: (name, t) -> True

            def pump():
                popped = 0
                if laneA and laneA[0][0] <= it[0]:
                    laneA.pop(0)[1]()
                    popped = 1
                for _ in range(2 - popped):
                    if laneB and laneB[0][0] <= it[0] and laneB[0][1]():
                        laneB.pop(0)[2]()
                if laneC and laneC[0][0] <= it[0] and laneC[0][1]():
                    laneC.pop(0)[2]()
                it[0] += 1

            hold = {}

            # vh items: [128, 130] t-chunks; block j = t//4
            def vh_item(b, t, xv_of):
                def tt():
                    ps = pps.tile([128, 130], f32, tag="p1", name=f"pvh{b}{t}")
                    for d in range(ND):
                        nc.tensor.matmul(ps, xv_of(d, t), wqvd(d),
                                         start=(d == 0), stop=(d == ND - 1))
                    nc.vector.tensor_add(hold["vh" + str(b)][:, t, :],
                                         ps[:], bqv_bc[:])
                    done[("vh" + str(b), t)] = True
                return tt

            def xv_slice(blocks, d, t):
                c = d * 512 + (t % 4) * 128
                return blocks[t // 4][:, c:c + 128]

            def xv0_of(d, t):
                return xv_slice(vx0, d, t)

            def xv1_of(d, t):
                return xv_slice(hold["vt1"], d, t)

            # b1 q/k block loads + projections as lane-A items.  Each b1
            # block DMA reuses an xg buf whose b0 reader must already be
            # EMITTED (Tile WAR deps only see emitted readers), so each
            # load is its own item gated just past that reader's item.
            def qk_chain_thunks(base, step):
                items = []

                def alloc_thunk():
                    hold["qt1"] = [None] * NSC
                    hold["kt1"] = [None] * NSC
                    hold["qh"] = pproj.tile([128, S], bf16, tag="projq",
                                            name="projq1")
                    hold["kh"] = pproj.tile([128, S], bf16, tag="projk",
                                            name="projk1")
                    hold["qt1"][0] = dma_blk(qT, 1, 0, "q")
                    hold["qt1"][1] = dma_blk(qT, 1, 1, "q")
                items.append((1, alloc_thunk))

                def b1_dma(tgt, src, j, nm):
                    def th():
                        hold[tgt][j] = dma_blk(src, 1, j, nm)
                    return th
                items.append((6, b1_dma("qt1", qT, 2, "q")))
                items.append((9, b1_dma("qt1", qT, 3, "q")))
                items.append((11, b1_dma("kt1", kT, 0, "k")))
                items.append((13, b1_dma("kt1", kT, 1, "k")))
                items.append((16, b1_dma("kt1", kT, 2, "k")))
                items.append((18, b1_dma("kt1", kT, 3, "k")))
                cell = {}
                for i, name in enumerate(("q", "k")):
                    for j in range(NSC):
                        def t1a(name=name, j=j):
                            ps = pps.tile([128, 512], f32,
                                          tag="p1", name=f"pb{name}{j}")
                            xt = hold["qt1" if name == "q" else "kt1"][j]
                            for d in range(4):
                                nc.tensor.matmul(
                                    ps, wqd(d), xt[:, d * 512:(d + 1) * 512],
                                    start=(d == 0), stop=False)
                            cell[(name, j)] = ps

                        def t1b(name=name, j=j):
                            ps = cell[(name, j)]
                            xt = hold["qt1" if name == "q" else "kt1"][j]
                            for d in range(4, ND):
                                nc.tensor.matmul(
                                    ps, wqd(d), xt[:, d * 512:(d + 1) * 512],
                                    start=False,
                                    stop=(d == ND - 1))
                            sb = hold["qh" if name == "q" else "kh"]
                            nc.vector.tensor_scalar_add(
                                sb[:, j * 512:(j + 1) * 512], ps, bq_col[:])
                        items.append((base[i] + step * j, t1a))
                        items.append((base[i] + step * j, t1b))
                return items

            f32r = mybir.dt.float32r

            def norm_rec_a(att, sfx, den_eng=None):
                # den rows copied to partition 0 first: the custom-DVE
                # reciprocal mislowers partition-offset inputs on HW.
                den = psmall.tile([1, 2, 512], f32, tag="dcp", bufs=1,
                                  name="den" + sfx)
                nc.vector.tensor_copy(den[0:1, 0, :], att[0][64:65, :])
                if den_eng is None:
                    nc.vector.tensor_copy(den[0:1, 1, :], att[1][64:65, :])
                else:
                    den_eng.copy(den[0:1, 1, :], att[1][64:65, :])
                rec = psmall.tile([1, 2, 512], f32, tag="den", bufs=1,
                                  name="rec" + sfx)
                nc.vector.reciprocal_approx_fast(rec[:], den[:])
                recb = psmall.tile([1, 2, 512], bf16, tag="recb", bufs=1,
                                   name="recb" + sfx)
                nc.vector.tensor_copy(recb[:], rec[:])
                return recb

            def norm_rec_b(recb, sfx):
                # per-head K=1 broadcast matmuls against the sel2 ones row
                bcds = []
                for h in range(HPC):
                    bcd = pps.tile([64, 512], f32, tag="p1",
                                   name=f"bcd{h}" + sfx)
                    nc.tensor.matmul(bcd, sel2[0:1, 0:64], recb[0:1, h, :],
                                     start=True, stop=True)
                    bcds.append(bcd)
                return bcds

            def norm_v2(att, oT, ssl, sfx):
                # deferred path: att is sbuf (asb); muls read bcd psum
                bcds = norm_rec_b(norm_rec_a(att, sfx), sfx)
                for h in range(HPC):
                    hp = slice(h * 64, (h + 1) * 64)
                    nc.vector.tensor_mul(oT[hp, ssl],
                                         att[h][0:64, :], bcds[h][:, :])

            def outproj_s1(b, sc, s1, oT, sfx, cast_eng=None,
                           split_store=False):
                s0 = sc * 512 + s1 * 128
                rs = slice(b * S + s0, b * S + s0 + 128)
                ob = pob.tile([128, D], bf16, tag="ob", name="ob" + sfx)
                # b0 stores ride the gpsimd software queue; b1 stores ride
                # sync (free after the b1 loads) so the tail drains fast
                seng = nc.gpsimd if b == 0 else nc.sync
                for n in range(2):
                    nsl = slice(n * 512, (n + 1) * 512)
                    ps = pps.tile([128, 512], f32, tag="p1",
                                  name="opps" + sfx)
                    nc.tensor.matmul(ps, oT[:, s0:s0 + 128], wo_sb[:, nsl],
                                     start=True, stop=True)
                    if cast_eng is None:
                        nc.vector.tensor_copy(ob[:, nsl], ps)
                    else:
                        cast_eng.copy(ob[:, nsl], ps)
                    if split_store:
                        # both halves on the sync hw queue: a gpsimd-queue
                        # store here adds a ~4us software-DGE drain tail
                        nc.sync.dma_start(out[rs, nsl], ob[:, nsl])
                if not split_store:
                    seng.dma_start(out[rs, :], ob[:])

            def defer_attnv(b, sc, exs, vh_of, oT, gate, og):
                ssl = slice(sc * 512, (sc + 1) * 512)
                cell = {}
                for t in range(NT):
                    def av(t=t):
                        if t == 0:
                            cell["att"] = [
                                pps.tile([65, 512], f32, tag="att",
                                         name=f"att{b}{sc}{h}")
                                for h in range(HPC)]
                        vh = vh_of()
                        for h in range(HPC):
                            nc.tensor.matmul(cell["att"][h],
                                             vh[:, t, h * 65:h * 65 + 65],
                                             exs[t][:, h, :],
                                             start=(t == 0), stop=(t == NT - 1))
                    g = gate(t) if gate is not None else 0
                    laneB.append((g,
                                  (lambda t=t: ("vh" + str(b), t) in done),
                                  av))

                # asb copies (DVE-only) free the att psum bank promptly so
                # the next s-chunk's attn@V can start; the norm matmul +
                # out-proj go to laneC, gated past the last attn@V so score
                # matmuls sit between them in the in-order PE queue, hiding
                # the DVE reciprocal-chain latency.
                def asb_copy():
                    cell["asb"] = [
                        psmall.tile([65, 512], f32, tag="asb", bufs=4,
                                    name=f"asb{b}{sc}{h}")
                        for h in range(HPC)]
                    for h in range(HPC):
                        nc.vector.tensor_copy(cell["asb"][h][:],
                                              cell["att"][h][:])
                laneB.append((0, lambda: True, asb_copy))
                d0 = it[0]

                def norm():
                    norm_v2(cell["asb"], oT, ssl, f"{b}{sc}")
                    cell["normed"] = True
                laneC.append((d0 + 13, lambda: "asb" in cell, norm))

                for g in range(2):
                    def op(g=g):
                        for u in range(2):
                            outproj_s1(b, sc, g * 2 + u, oT, f"{b}{sc}")
                    laneC.append((og[g], lambda: "normed" in cell, op))

            def inline_tail(b, sc, att, oT):
                # att is psum here.  den-h1 copy rides ScalarE (exps are
                # done; the COPY table is shared with the ob casts), bcs
                # copies keep the muls off dual-psum reads, and each
                # u-chunk's muls feed its outproj immediately.  The last
                # deferred outproj group is drained between the norm
                # phases: real PE work covering the DVE reciprocal chain.
                ssl = slice(sc * 512, (sc + 1) * 512)
                recb = norm_rec_a(att, "L", den_eng=nc.scalar)
                # drain the held-back deferred outproj group here: its 4
                # matmuls fill the PE queue while DVE runs den->rec->recb
                if laneC and laneC[0][1]():
                    laneC.pop(0)[2]()
                bcds = norm_rec_b(recb, "L")
                bcss = []
                for h in range(HPC):
                    bcs = psmall.tile([64, 512], f32, tag="bcs", bufs=1,
                                      name=f"bcs{h}L")
                    nc.vector.tensor_copy(bcs[:], bcds[h][:, :])
                    bcss.append(bcs)
                for u in range(4):
                    us = slice(u * 128, (u + 1) * 128)
                    for h in range(HPC):
                        hp = slice(h * 64, (h + 1) * 64)
                        nc.vector.tensor_mul(oT[hp, ssl][:, us],
                                             att[h][0:64, us], bcss[h][:, us])
                    eng = nc.scalar if u % 2 else None
                    outproj_s1(b, sc, u, oT, "L", cast_eng=eng,
                               split_store=(u == 3))

            def attention(b, qh, kh, vh_of, gate0=None, last=False):
                oT = poutT.tile([128, S], bf16, tag="outT", name=f"oT{b}")
                for sc in range(NSC):
                    inline = last and sc == NSC - 1
                    exs = []
                    att = None
                    for t in range(NT):
                        pump()
                        scps = pps.tile([128, HPC, 512], f32, tag="sc")
                        for h in range(HPC):
                            hp = slice(h * 64, (h + 1) * 64)
                            nc.tensor.matmul(scps[:, h, :],
                                             kh[hp, t * 128:(t + 1) * 128],
                                             qh[hp, sc * 512:(sc + 1) * 512],
                                             start=True, stop=True)
                        ex = pexp.tile([128, HPC, 512], bf16, tag="exp")
                        nc.scalar.activation(ex[:], scps[:], Exp, scale=0.125)
                        exs.append(ex)
                        if inline:
                            if att is None:
                                att = [pps.tile([65, 512], f32, tag="att",
                                                name=f"attL{h}")
                                       for h in range(HPC)]
                            vh = vh_of()
                            for h in range(HPC):
                                nc.tensor.matmul(att[h],
                                                 vh[:, t, h * 65:h * 65 + 65],
                                                 ex[:, h, :],
                                                 start=(t == 0),
                                                 stop=(t == NT - 1))
                    if inline:
                        inline_tail(b, sc, att, oT)
                    else:
                        gate = gate0 if (gate0 is not None and sc == 0) \
                            else None
                        # outproj spread through the next window; b1-sc2's
                        # goes to the inline tail's manual drain (129/130
                        # never pop in-loop)
                        # b1-sc2's second outproj group is held back (999)
                        # for the inline tail's manual pop: real PE work
                        # covering the DVE reciprocal chain
                        idx = b * NSC + sc
                        og = (it[0] + 14, it[0] + 15) if idx < 6 \
                            else (it[0] + 14, 999)
                        defer_attnv(b, sc, exs, vh_of, oT, gate, og)

            # ---- lane-A schedule ----
            # iters are scores-iterations (~1.1us each from ~13us).
            # landings (sync hw queue, ramping ~170->280GB/s): k j1/2/3
            # ~18/21/23us -> kh-j at 5/8/10; q j1..3 ~26/29/31 -> qh-j at
            # 12/15/17; v0 j-blocks ~34/37/40/43 -> vh t at 19+t; b1 q1
            # ~46-55 -> 30+3j, k1 ~58-67 -> 41+3j, v1 ~70-79 -> vh1 52+t.
            hold["vh0"] = pvh.tile([128, NT, 130], bf16, tag="vh", name="vh0")
            items = [(4, lambda: proj_j(kx0[1], kh0, 1)),
                     (8, lambda: proj_j(kx0[2], kh0, 2)),
                     (10, lambda: proj_j(kx0[3], kh0, 3)),
                     (12, lambda: proj_j(qx0[1], qh0, 1)),
                     (15, lambda: proj_j(qx0[2], qh0, 2)),
                     (17, lambda: proj_j(qx0[3], qh0, 3))]
            for t in range(NT):
                items.append((19 + t, vh_item(0, t, xv0_of)))
            items += qk_chain_thunks((30, 41), 3)

            def v1_first():
                hold["vt1"] = [None] * NSC
                hold["vh1"] = pvh.tile([128, NT, 130], bf16, tag="vh",
                                       name="vh1")
                hold["vt1"][0] = dma_blk(vT, 1, 0, "v")
            items.append((23, v1_first))
            for j in range(1, NSC):
                def v1_blk(j=j):
                    hold["vt1"][j] = dma_blk(vT, 1, j, "v")
                items.append((23 + 4 * j, v1_blk))
            for t in range(NT):
                items.append((52 + t, vh_item(1, t, xv1_of)))
            laneA.extend(sorted(items, key=lambda x: x[0]))

            def gate_b0sc0(t):
                return 19 + t

            attention(0, qh0, kh0, lambda: hold["vh0"], gate0=gate_b0sc0)
            attention(1, hold["qh"], hold["kh"], lambda: hold["vh1"],
                      last=True)

            while laneA or laneB or laneC:
                progressed = False
                if laneA:
                    laneA.pop(0)[1]()
                    progressed = True
                if laneB and (laneB[0][1]() or not progressed):
                    laneB.pop(0)[2]()
                    progressed = True
                if laneC and (laneC[0][1]() or not progressed):
                    laneC.pop(0)[2]()

    nc.compile()
    return nc


def make_in_maps(q, k, v, Wq, bq, Wo):
    bf = ml_dtypes.bfloat16
    xT = {}
    for name, x in (("qT", q), ("kT", k), ("vT", v)):
        # per-(batch, j) blocks [128, ND*512]: block (b, j) holds
        # x[b, j*512:(j+1)*512, :] with layout [p, d*512 + c] =
        # x[b, j*512 + c, d*128 + p] -> 8KB contiguous per partition
        xa = np.asarray(x, np.float32).reshape(B, NSC, 512, ND, 128)
        xT[name] = np.ascontiguousarray(
            xa.transpose(4, 0, 1, 3, 2).reshape(128, B * NSC * BLK)
        ).astype(bf)

    in_maps = []
    for c in range(NCORES):
        cols = slice(c * HD, (c + 1) * HD)
        wqc = np.asarray(Wq, np.float32)[:, cols]
        bqc = np.asarray(bq, np.float32)[cols]
        wqve = np.zeros((D, 130), np.float32)
        wqve[:, 0:64] = wqc[:, 0:64]
        wqve[:, 65:129] = wqc[:, 64:128]
        bqve = np.zeros((1, 130), np.float32)
        bqve[0, 0:64] = bqc[0:64]
        bqve[0, 65:129] = bqc[64:128]
        bqve[0, 64] = 1.0
        bqve[0, 129] = 1.0
        # d-major packing: wq_pk[p, d*HD+c] = wqc[d*128+p, c]
        wq_pk = np.ascontiguousarray(
            wqc.reshape(ND, 128, HD).transpose(1, 0, 2).reshape(128, ND * HD))
        wqv_pk = np.ascontiguousarray(
            wqve.reshape(ND, 128, 130).transpose(1, 0, 2).reshape(128, ND * 130))
        sel2 = np.zeros((2, 128), np.float32)
        sel2[0, 0:64] = 1.0
        sel2[1, 64:128] = 1.0
        in_maps.append({
            "qT": xT["qT"], "kT": xT["kT"], "vT": xT["vT"],
            "wq": wq_pk.astype(bf),
            "wqv": wqv_pk.astype(bf),
            "bqc": np.ascontiguousarray(bqc[:, None]),
            "bqvb": np.ascontiguousarray(np.tile(bqve, (128, 1))),
            "wo": np.ascontiguousarray(np.asarray(Wo, np.float32)[cols, :]).astype(bf),
            "sel2": sel2.astype(bf),
        })
    return in_maps


def kernel(q, k, v, Wq, bq, Wo, bo):
    import jax
    from concourse.bass_utils import run_bass_kernel_spmd

    try:
        jax.config.update("jax_compilation_cache_dir", "/tmp/jax_bass_cache")
        jax.config.update("jax_persistent_cache_min_entry_size_bytes", -1)
        jax.config.update("jax_persistent_cache_min_compile_time_secs", 0)
    except Exception:
        pass

    if "nc" not in _cache:
        _cache["nc"] = _build()
    nc = _cache["nc"]

    in_maps = make_in_maps(q, k, v, Wq, bq, Wo)
    res = run_bass_kernel_spmd(nc, in_maps, list(range(NCORES)), trace=False)
    acc = np.zeros((BS, D), np.float64)
    for c in range(NCORES):
        acc += res.results[c]["out"].astype(np.float64)
    acc += np.asarray(bo, np.float32)[None, :].astype(np.float64)
    return acc.reshape(B, S, D).astype(np.float32)


# revision 65
# speedup vs baseline: 1.0482x; 1.0113x over previous
"""Multi-head attention (B=2, S=2048, D=1024, H=16, d_k=64) on 8 TRN2 NeuronCores.

Sharding: head-parallel. Core c owns heads (2c, 2c+1) for both batch rows:
 - replicated inputs: qT/kT/vT host-packed as per-(batch, 512-col j-block)
   blocks [128, ND*512] so each block is ONE [128, 4096] DMA with an 8KB
   contiguous run per partition (full hardware-DGE rate, cheap trigger);
   within a block the 8 d-chunks are column-major groups, d on partitions
   so the TensorEngine contracts over D with no transposes.
 - per-core weights: Wq columns / Wo rows for its two heads (host pre-packs
   wq/wqv d-major so each is ONE contiguous DMA)
 - per-core output: partial = attn_out(own heads) @ Wo[own rows]  [4096, 1024] bf16
   The host sums the 8 partials (f32) and adds bo.  No cross-core comm.

Per-core dataflow (bf16 matmuls, f32 PSUM):
 1. wq + bq + all x blocks ride the sync hardware-DMA queue (the gpsimd
    software queue starts ~10us late and runs ~3x slower -- only the
    late-needed consts and b0 stores go there).  Sync order: q(b0,j0),
    k(b0,j0..3), q(b0,j1..3), v(b0,j0..3), then b1 q/k/v via lane-A
    thunks.  qh-j0 projects as soon as its block lands; kh j0..3
    projections chase their DMAs as lane-A items inside the t-loop, so
    scores(sc0) start ~7us after the first matmul instead of waiting for
    all of k.
 2. qhT/khT [128, 2048] per batch = Wq_c.T @ xT (+bq).  vh [2048, 130]
    natural = vT.T @ Wqv_c; Wqv has zero-cols / bqv has 1.0-cols so each
    head gets a ones column -> attn@V also produces softmax denominators.
 3. scoresT[t,s] = khT.T @ qhT, both heads packed into disjoint PE
    row-groups (K=64).  exp(x/8) on ScalarE from PSUM, bf16 out.
 4. attn@V accumulated over t; row 64 = denominator.  Normalize: DVE
    fast-reciprocal straight off the two denominator rows into a [1,2,512]
    f32 tile, bf16 rounding copy, per-head K=1 matmuls against the sel2
    ones row broadcast the reciprocals across partitions, then DVE muls
    (att sbuf x bcast psum) produce normalized oT bf16.
 5. partial[s, :] = outT.T @ Wo_c -> bf16 ob [128,1024] (two PSUM copies)
    -> ONE DRAM store per 128 rows.  b0 stores ride gpsimd; b1 stores ride
    sync (free after the b1 loads), so the tail drain is short.

Scheduling: ScalarE (exp, ~143us) and the TensorEngine (~170us execute) --
PE is the bottleneck, so emission keeps the PE queue dense: each s-chunk's
scores+exp loop is emitted first; its attn@V/normalize/out-proj are
deferred one s-chunk and re-emitted between later score iterations via a
three-lane work queue (lane A: DMA-gated projection work with
earliest-iteration thresholds; lane B: deferred attention work, also
min-iter gated so no PE instruction is emitted before its input DMA can
have landed -- the PE queue is in-order, a stalled instruction blocks it).
"""

import numpy as np
import ml_dtypes

B, S, D, H, DK = 2, 2048, 1024, 16, 64
NCORES = 8
HPC = H // NCORES          # heads per core = 2
BS = B * S                 # 4096
HD = HPC * DK              # 128 = per-core head dims
ND = D // 128              # 8 d-chunks
NSC = S // 512             # 4 column blocks per batch
BLK = ND * 512             # 4096 elems per partition per block

_cache = {}


def _build():
    import concourse.bass as bass
    import concourse.tile as tile
    from concourse import bacc, mybir

    f32 = mybir.dt.float32
    bf16 = mybir.dt.bfloat16
    Exp = mybir.ActivationFunctionType.Exp

    nc = bacc.Bacc("TRN2", target_bir_lowering=False, debug=False,
                   num_devices=NCORES)

    qT = nc.declare_dram_parameter("qT", [128, B * NSC * BLK], bf16,
                                   isOutput=False)
    kT = nc.declare_dram_parameter("kT", [128, B * NSC * BLK], bf16,
                                   isOutput=False)
    vT = nc.declare_dram_parameter("vT", [128, B * NSC * BLK], bf16,
                                   isOutput=False)
    wq = nc.declare_dram_parameter("wq", [128, ND * HD], bf16, isOutput=False)
    wqv = nc.declare_dram_parameter("wqv", [128, ND * 130], bf16, isOutput=False)
    bqc = nc.declare_dram_parameter("bqc", [HD, 1], f32, isOutput=False)
    bqvb = nc.declare_dram_parameter("bqvb", [128, 130], f32, isOutput=False)
    wo = nc.declare_dram_parameter("wo", [HD, D], bf16, isOutput=False)
    sel2d = nc.declare_dram_parameter("sel2", [2, 128], bf16, isOutput=False)
    out = nc.declare_dram_parameter("out", [BS, D], bf16, isOutput=True)

    NT = S // 128            # 16 t-chunks per batch

    with tile.TileContext(nc) as tc:
        with (
            tc.tile_pool(name="const", bufs=1) as pc,
            tc.tile_pool(name="xg", bufs=12) as pin1,
            tc.tile_pool(name="proj", bufs=2) as pproj,
            tc.tile_pool(name="vh", bufs=2) as pvh,
            tc.tile_pool(name="exp", bufs=19) as pexp,
            tc.tile_pool(name="outT", bufs=2) as poutT,
            tc.tile_pool(name="small", bufs=2) as psmall,
            tc.tile_pool(name="ob", bufs=3) as pob,
            tc.tile_pool(name="ps", bufs=2, space="PSUM") as pps,
        ):
            # ---- sync hardware queue: wq + bq first (first proj needs
            # them), then q j0, k j0..3, q j1..3, v j0..3
            wq_sb = pc.tile([128, ND * HD], bf16)
            nc.sync.dma_start(wq_sb[:], wq[:, :])
            bq_col = pc.tile([128, 1], f32)
            nc.sync.dma_start(bq_col[:], bqc[:, :])

            def dma_blk(src, b, j, nm):
                t = pin1.tile([128, BLK], bf16, tag="xg",
                              name=f"x{nm}{b}{j}")
                off = (b * NSC + j) * BLK
                nc.sync.dma_start(t[:], src[:, off:off + BLK])
                return t

            qx0 = [None] * NSC
            kx0 = [None] * NSC
            vx0 = [None] * NSC
            qx0[0] = dma_blk(qT, 0, 0, "q")
            # k's first block is host-packed as two t-halves so kh-j0a
            # (t 0:256) lands 1MB earlier than the full block would and
            # scores t0/t1 start sooner
            kx0h = []
            for hblk in range(2):
                t = pin1.tile([128, BLK // 2], bf16, tag="xg",
                              name=f"xk00{hblk}")
                nc.sync.dma_start(
                    t[:], kT[:, hblk * (BLK // 2):(hblk + 1) * (BLK // 2)])
                kx0h.append(t)
            for j in range(1, NSC):
                kx0[j] = dma_blk(kT, 0, j, "k")
            for j in range(1, NSC):
                qx0[j] = dma_blk(qT, 0, j, "q")
            for j in range(NSC):
                vx0[j] = dma_blk(vT, 0, j, "v")

            # ---- gpsimd software queue: only late-needed consts
            bqv_bc = pc.tile([128, 130], f32)
            nc.gpsimd.dma_start(bqv_bc[:], bqvb[:, :])
            wqv_sb = pc.tile([128, ND * 130], bf16)
            nc.gpsimd.dma_start(wqv_sb[:], wqv[:, :])
            wo_sb = pc.tile([HD, D], bf16)
            nc.gpsimd.dma_start(wo_sb[:], wo[:, :])
            # [2,128] bf16 selector (host-packed): row 0 is ones on cols
            # 0:64 (the K=1 broadcast stationary); row h is 1 on head h's
            # 64 cols
            sel2 = pc.tile([2, 128], bf16)
            nc.gpsimd.dma_start(sel2[:], sel2d[:, :])


            def wqd(d):
                return wq_sb[:, d * HD:(d + 1) * HD]

            def wqvd(d):
                return wqv_sb[:, d * 130:(d + 1) * 130]

            qh0 = pproj.tile([128, S], bf16, tag="projq", name="projq0")
            kh0 = pproj.tile([128, S], bf16, tag="projk", name="projk0")

            # one j-block projection: 8 accumulating matmuls + bias add
            def proj_j(xt, sb, j):
                ps = pps.tile([128, 512], f32, tag="p1", name=f"pj{j}")
                for d in range(ND):
                    nc.tensor.matmul(ps, wqd(d),
                                     xt[:, d * 512:(d + 1) * 512],
                                     start=(d == 0), stop=(d == ND - 1))
                nc.vector.tensor_scalar_add(
                    sb[:, j * 512:(j + 1) * 512], ps, bq_col[:])

            # half-block projection: 8 accumulating N=256 matmuls + add
            def proj_jh(xt, sb, hblk):
                ps = pps.tile([128, 256], f32, tag="p1", name=f"pjh{hblk}")
                for d in range(ND):
                    nc.tensor.matmul(ps, wqd(d),
                                     xt[:, d * 256:(d + 1) * 256],
                                     start=(d == 0), stop=(d == ND - 1))
                nc.vector.tensor_scalar_add(
                    sb[:, hblk * 256:(hblk + 1) * 256], ps, bq_col[:])

            # qh j0 + kh j0a before the t-loop; kh j0b + j1..3 chase
            # their DMAs as lane-A items inside it.
            proj_j(qx0[0], qh0, 0)
            proj_jh(kx0h[0], kh0, 0)

            # ---- three-lane deferred work queue ----
            laneA = []   # (min_iter, thunk): DMA-gated projection work
            laneB = []   # (min_iter, ready_fn, thunk): attn@V + asb copies
            laneC = []   # (min_iter, ready_fn, thunk): norm + out-proj
            it = [0]
            done = {}    # emission flags: (name, t) -> True

            def pump():
                popped = 0
                if laneA and laneA[0][0] <= it[0]:
                    laneA.pop(0)[1]()
                    popped = 1
                for _ in range(2 - popped):
                    if laneB and laneB[0][0] <= it[0] and laneB[0][1]():
                        laneB.pop(0)[2]()
                if laneC and laneC[0][0] <= it[0] and laneC[0][1]():
                    laneC.pop(0)[2]()
                it[0] += 1

            hold = {}

            # vh items: [128, 130] t-chunks; block j = t//4
            def vh_item(b, t, xv_of):
                def tt():
                    ps = pps.tile([128, 130], f32, tag="p1", name=f"pvh{b}{t}")
                    for d in range(ND):
                        nc.tensor.matmul(ps, xv_of(d, t), wqvd(d),
                                         start=(d == 0), stop=(d == ND - 1))
                    nc.vector.tensor_add(hold["vh" + str(b)][:, t, :],
                                         ps[:], bqv_bc[:])
                    done[("vh" + str(b), t)] = True
                return tt

            def xv_slice(blocks, d, t):
                c = d * 512 + (t % 4) * 128
                return blocks[t // 4][:, c:c + 128]

            def xv0_of(d, t):
                return xv_slice(vx0, d, t)

            def xv1_of(d, t):
                return xv_slice(hold["vt1"], d, t)

            # b1 q/k block loads + projections as lane-A items.  Each b1
            # block DMA reuses an xg buf whose b0 reader must already be
            # EMITTED (Tile WAR deps only see emitted readers), so each
            # load is its own item gated just past that reader's item.
            def qk_chain_thunks(base, step):
                items = []

                def alloc_thunk():
                    hold["qt1"] = [None] * NSC
                    hold["kt1"] = [None] * NSC
                    hold["qh"] = pproj.tile([128, S], bf16, tag="projq",
                                            name="projq1")
                    hold["kh"] = pproj.tile([128, S], bf16, tag="projk",
                                            name="projk1")
                    hold["qt1"][0] = dma_blk(qT, 1, 0, "q")
                    hold["qt1"][1] = dma_blk(qT, 1, 1, "q")
                # iter 3: q11 reuses k00b's buf, whose reader (kh-j0b)
                # is emitted at iter 2
                items.append((3, alloc_thunk))

                def b1_dma(tgt, src, j, nm):
                    def th():
                        hold[tgt][j] = dma_blk(src, 1, j, nm)
                    return th
                items.append((6, b1_dma("qt1", qT, 2, "q")))
                items.append((9, b1_dma("qt1", qT, 3, "q")))
                items.append((11, b1_dma("kt1", kT, 0, "k")))
                items.append((13, b1_dma("kt1", kT, 1, "k")))
                items.append((16, b1_dma("kt1", kT, 2, "k")))
                items.append((18, b1_dma("kt1", kT, 3, "k")))
                cell = {}
                for i, name in enumerate(("q", "k")):
                    for j in range(NSC):
                        def t1a(name=name, j=j):
                            ps = pps.tile([128, 512], f32,
                                          tag="p1", name=f"pb{name}{j}")
                            xt = hold["qt1" if name == "q" else "kt1"][j]
                            for d in range(4):
                                nc.tensor.matmul(
                                    ps, wqd(d), xt[:, d * 512:(d + 1) * 512],
                                    start=(d == 0), stop=False)
                            cell[(name, j)] = ps

                        def t1b(name=name, j=j):
                            ps = cell[(name, j)]
                            xt = hold["qt1" if name == "q" else "kt1"][j]
                            for d in range(4, ND):
                                nc.tensor.matmul(
                                    ps, wqd(d), xt[:, d * 512:(d + 1) * 512],
                                    start=False,
                                    stop=(d == ND - 1))
                            sb = hold["qh" if name == "q" else "kh"]
                            nc.vector.tensor_scalar_add(
                                sb[:, j * 512:(j + 1) * 512], ps, bq_col[:])
                        items.append((base[i] + step * j, t1a))
                        items.append((base[i] + step * j, t1b))
                return items

            f32r = mybir.dt.float32r

            def norm_rec_a(att, sfx, den_eng=None):
                # den rows copied to partition 0 first: the custom-DVE
                # reciprocal mislowers partition-offset inputs on HW.
                den = psmall.tile([1, 2, 512], f32, tag="dcp", bufs=1,
                                  name="den" + sfx)
                nc.vector.tensor_copy(den[0:1, 0, :], att[0][64:65, :])
                if den_eng is None:
                    nc.vector.tensor_copy(den[0:1, 1, :], att[1][64:65, :])
                else:
                    den_eng.copy(den[0:1, 1, :], att[1][64:65, :])
                rec = psmall.tile([1, 2, 512], f32, tag="den", bufs=1,
                                  name="rec" + sfx)
                nc.vector.reciprocal_approx_fast(rec[:], den[:])
                recb = psmall.tile([1, 2, 512], bf16, tag="recb", bufs=1,
                                   name="recb" + sfx)
                nc.vector.tensor_copy(recb[:], rec[:])
                return recb

            def norm_rec_b(recb, sfx):
                # per-head K=1 broadcast matmuls against the sel2 ones row
                bcds = []
                for h in range(HPC):
                    bcd = pps.tile([64, 512], f32, tag="p1",
                                   name=f"bcd{h}" + sfx)
                    nc.tensor.matmul(bcd, sel2[0:1, 0:64], recb[0:1, h, :],
                                     start=True, stop=True)
                    bcds.append(bcd)
                return bcds

            def norm_v2(att, oT, ssl, sfx):
                # deferred path: att is sbuf (asb); muls read bcd psum
                bcds = norm_rec_b(norm_rec_a(att, sfx), sfx)
                for h in range(HPC):
                    hp = slice(h * 64, (h + 1) * 64)
                    nc.vector.tensor_mul(oT[hp, ssl],
                                         att[h][0:64, :], bcds[h][:, :])

            def outproj_s1(b, sc, s1, oT, sfx, cast_eng=None,
                           split_store=False):
                s0 = sc * 512 + s1 * 128
                rs = slice(b * S + s0, b * S + s0 + 128)
                ob = pob.tile([128, D], bf16, tag="ob", name="ob" + sfx)
                # b0 stores ride the gpsimd software queue; b1 stores ride
                # sync (free after the b1 loads) so the tail drains fast
                seng = nc.gpsimd if b == 0 else nc.sync
                for n in range(2):
                    nsl = slice(n * 512, (n + 1) * 512)
                    ps = pps.tile([128, 512], f32, tag="p1",
                                  name="opps" + sfx)
                    nc.tensor.matmul(ps, oT[:, s0:s0 + 128], wo_sb[:, nsl],
                                     start=True, stop=True)
                    if cast_eng is None:
                        nc.vector.tensor_copy(ob[:, nsl], ps)
                    else:
                        cast_eng.copy(ob[:, nsl], ps)
                    if split_store:
                        # both halves on the sync hw queue: a gpsimd-queue
                        # store here adds a ~4us software-DGE drain tail
                        nc.sync.dma_start(out[rs, nsl], ob[:, nsl])
                if not split_store:
                    seng.dma_start(out[rs, :], ob[:])

            def defer_attnv(b, sc, exs, vh_of, oT, gate, og):
                ssl = slice(sc * 512, (sc + 1) * 512)
                cell = {}
                for t in range(NT):
                    def av(t=t):
                        if t == 0:
                            cell["att"] = [
                                pps.tile([65, 512], f32, tag="att",
                                         name=f"att{b}{sc}{h}")
                                for h in range(HPC)]
                        vh = vh_of()
                        for h in range(HPC):
                            nc.tensor.matmul(cell["att"][h],
                                             vh[:, t, h * 65:h * 65 + 65],
                                             exs[t][:, h, :],
                                             start=(t == 0), stop=(t == NT - 1))
                    g = gate(t) if gate is not None else 0
                    laneB.append((g,
                                  (lambda t=t: ("vh" + str(b), t) in done),
                                  av))

                # asb copies (DVE-only) free the att psum bank promptly so
                # the next s-chunk's attn@V can start; the norm matmul +
                # out-proj go to laneC, gated past the last attn@V so score
                # matmuls sit between them in the in-order PE queue, hiding
                # the DVE reciprocal-chain latency.
                def asb_copy():
                    cell["asb"] = [
                        psmall.tile([65, 512], f32, tag="asb", bufs=4,
                                    name=f"asb{b}{sc}{h}")
                        for h in range(HPC)]
                    for h in range(HPC):
                        nc.vector.tensor_copy(cell["asb"][h][:],
                                              cell["att"][h][:])
                laneB.append((0, lambda: True, asb_copy))
                d0 = it[0]

                def norm():
                    norm_v2(cell["asb"], oT, ssl, f"{b}{sc}")
                    cell["normed"] = True
                laneC.append((d0 + 13, lambda: "asb" in cell, norm))

                for g in range(2):
                    def op(g=g):
                        for u in range(2):
                            outproj_s1(b, sc, g * 2 + u, oT, f"{b}{sc}")
                    laneC.append((og[g], lambda: "normed" in cell, op))

            def inline_tail(b, sc, att, oT):
                # att is psum here.  den-h1 copy rides ScalarE (exps are
                # done; the COPY table is shared with the ob casts), bcs
                # copies keep the muls off dual-psum reads, and each
                # u-chunk's muls feed its outproj immediately.  The last
                # deferred outproj group is drained between the norm
                # phases: real PE work covering the DVE reciprocal chain.
                ssl = slice(sc * 512, (sc + 1) * 512)
                recb = norm_rec_a(att, "L", den_eng=nc.scalar)
                # drain the held-back deferred outproj group here: its 4
                # matmuls fill the PE queue while DVE runs den->rec->recb
                if laneC and laneC[0][1]():
                    laneC.pop(0)[2]()
                bcds = norm_rec_b(recb, "L")
                bcss = []
                for h in range(HPC):
                    bcs = psmall.tile([64, 512], f32, tag="bcs", bufs=1,
                                      name=f"bcs{h}L")
                    nc.vector.tensor_copy(bcs[:], bcds[h][:, :])
                    bcss.append(bcs)
                for u in range(4):
                    us = slice(u * 128, (u + 1) * 128)
                    for h in range(HPC):
                        hp = slice(h * 64, (h + 1) * 64)
                        nc.vector.tensor_mul(oT[hp, ssl][:, us],
                                             att[h][0:64, us], bcss[h][:, us])
                    eng = nc.scalar if u % 2 else None
                    outproj_s1(b, sc, u, oT, "L", cast_eng=eng,
                               split_store=(u == 3))

            def attention(b, qh, kh, vh_of, gate0=None, last=False):
                oT = poutT.tile([128, S], bf16, tag="outT", name=f"oT{b}")
                for sc in range(NSC):
                    inline = last and sc == NSC - 1
                    exs = []
                    att = None
                    for t in range(NT):
                        pump()
                        scps = pps.tile([128, HPC, 512], f32, tag="sc")
                        for h in range(HPC):
                            hp = slice(h * 64, (h + 1) * 64)
                            nc.tensor.matmul(scps[:, h, :],
                                             kh[hp, t * 128:(t + 1) * 128],
                                             qh[hp, sc * 512:(sc + 1) * 512],
                                             start=True, stop=True)
                        ex = pexp.tile([128, HPC, 512], bf16, tag="exp")
                        nc.scalar.activation(ex[:], scps[:], Exp, scale=0.125)
                        exs.append(ex)
                        if inline:
                            if att is None:
                                att = [pps.tile([65, 512], f32, tag="att",
                                                name=f"attL{h}")
                                       for h in range(HPC)]
                            vh = vh_of()
                            for h in range(HPC):
                                nc.tensor.matmul(att[h],
                                                 vh[:, t, h * 65:h * 65 + 65],
                                                 ex[:, h, :],
                                                 start=(t == 0),
                                                 stop=(t == NT - 1))
                    if inline:
                        inline_tail(b, sc, att, oT)
                    else:
                        gate = gate0 if (gate0 is not None and sc == 0) \
                            else None
                        # outproj spread through the next window; b1-sc2's
                        # goes to the inline tail's manual drain (129/130
                        # never pop in-loop)
                        # b1-sc2's second outproj group is held back (999)
                        # for the inline tail's manual pop: real PE work
                        # covering the DVE reciprocal chain
                        idx = b * NSC + sc
                        og = (it[0] + 14, it[0] + 15) if idx < 6 \
                            else (it[0] + 14, 999)
                        defer_attnv(b, sc, exs, vh_of, oT, gate, og)

            # ---- lane-A schedule ----
            # iters are scores-iterations (~1.1us each from ~13us).
            # landings (sync hw queue, ramping ~170->280GB/s): k j1/2/3
            # ~18/21/23us -> kh-j at 5/8/10; q j1..3 ~26/29/31 -> qh-j at
            # 12/15/17; v0 j-blocks ~34/37/40/43 -> vh t at 19+t; b1 q1
            # ~46-55 -> 30+3j, k1 ~58-67 -> 41+3j, v1 ~70-79 -> vh1 52+t.
            hold["vh0"] = pvh.tile([128, NT, 130], bf16, tag="vh", name="vh0")
            items = [(2, lambda: proj_jh(kx0h[1], kh0, 1)),
                     (4, lambda: proj_j(kx0[1], kh0, 1)),
                     (8, lambda: proj_j(kx0[2], kh0, 2)),
                     (10, lambda: proj_j(kx0[3], kh0, 3)),
                     (12, lambda: proj_j(qx0[1], qh0, 1)),
                     (15, lambda: proj_j(qx0[2], qh0, 2)),
                     (17, lambda: proj_j(qx0[3], qh0, 3))]
            for t in range(NT):
                items.append((19 + t, vh_item(0, t, xv0_of)))
            items += qk_chain_thunks((30, 41), 3)

            def v1_first():
                hold["vt1"] = [None] * NSC
                hold["vh1"] = pvh.tile([128, NT, 130], bf16, tag="vh",
                                       name="vh1")
                hold["vt1"][0] = dma_blk(vT, 1, 0, "v")
            items.append((23, v1_first))
            for j in range(1, NSC):
                def v1_blk(j=j):
                    hold["vt1"][j] = dma_blk(vT, 1, j, "v")
                items.append((23 + 4 * j, v1_blk))
            for t in range(NT):
                items.append((52 + t, vh_item(1, t, xv1_of)))
            laneA.extend(sorted(items, key=lambda x: x[0]))

            def gate_b0sc0(t):
                return 19 + t

            attention(0, qh0, kh0, lambda: hold["vh0"], gate0=gate_b0sc0)
            attention(1, hold["qh"], hold["kh"], lambda: hold["vh1"],
                      last=True)

            while laneA or laneB or laneC:
                progressed = False
                if laneA:
                    laneA.pop(0)[1]()
                    progressed = True
                if laneB and (laneB[0][1]() or not progressed):
                    laneB.pop(0)[2]()
                    progressed = True
                if laneC and (laneC[0][1]() or not progressed):
                    laneC.pop(0)[2]()

    nc.compile()
    return nc


def make_in_maps(q, k, v, Wq, bq, Wo):
    bf = ml_dtypes.bfloat16
    xT = {}
    for name, x in (("qT", q), ("kT", k), ("vT", v)):
        # per-(batch, j) blocks [128, ND*512]: block (b, j) holds
        # x[b, j*512:(j+1)*512, :] with layout [p, d*512 + c] =
        # x[b, j*512 + c, d*128 + p] -> 8KB contiguous per partition
        xa = np.asarray(x, np.float32).reshape(B, NSC, 512, ND, 128)
        P = xa.transpose(4, 0, 1, 3, 2).reshape(128, B * NSC * BLK).copy()
        if name == "kT":
            # first block split into two t-halves [p, half*2048 + d*256+c]
            # so kh-j0a can project before the full block lands
            sub = xa[0, 0].reshape(2, 256, ND, 128).transpose(3, 0, 2, 1)
            P[:, 0:BLK] = sub.reshape(128, BLK)
        xT[name] = np.ascontiguousarray(P).astype(bf)

    in_maps = []
    for c in range(NCORES):
        cols = slice(c * HD, (c + 1) * HD)
        wqc = np.asarray(Wq, np.float32)[:, cols]
        bqc = np.asarray(bq, np.float32)[cols]
        wqve = np.zeros((D, 130), np.float32)
        wqve[:, 0:64] = wqc[:, 0:64]
        wqve[:, 65:129] = wqc[:, 64:128]
        bqve = np.zeros((1, 130), np.float32)
        bqve[0, 0:64] = bqc[0:64]
        bqve[0, 65:129] = bqc[64:128]
        bqve[0, 64] = 1.0
        bqve[0, 129] = 1.0
        # d-major packing: wq_pk[p, d*HD+c] = wqc[d*128+p, c]
        wq_pk = np.ascontiguousarray(
            wqc.reshape(ND, 128, HD).transpose(1, 0, 2).reshape(128, ND * HD))
        wqv_pk = np.ascontiguousarray(
            wqve.reshape(ND, 128, 130).transpose(1, 0, 2).reshape(128, ND * 130))
        sel2 = np.zeros((2, 128), np.float32)
        sel2[0, 0:64] = 1.0
        sel2[1, 64:128] = 1.0
        in_maps.append({
            "qT": xT["qT"], "kT": xT["kT"], "vT": xT["vT"],
            "wq": wq_pk.astype(bf),
            "wqv": wqv_pk.astype(bf),
            "bqc": np.ascontiguousarray(bqc[:, None]),
            "bqvb": np.ascontiguousarray(np.tile(bqve, (128, 1))),
            "wo": np.ascontiguousarray(np.asarray(Wo, np.float32)[cols, :]).astype(bf),
            "sel2": sel2.astype(bf),
        })
    return in_maps


def kernel(q, k, v, Wq, bq, Wo, bo):
    import jax
    from concourse.bass_utils import run_bass_kernel_spmd

    try:
        jax.config.update("jax_compilation_cache_dir", "/tmp/jax_bass_cache")
        jax.config.update("jax_persistent_cache_min_entry_size_bytes", -1)
        jax.config.update("jax_persistent_cache_min_compile_time_secs", 0)
    except Exception:
        pass

    if "nc" not in _cache:
        _cache["nc"] = _build()
    nc = _cache["nc"]

    in_maps = make_in_maps(q, k, v, Wq, bq, Wo)
    res = run_bass_kernel_spmd(nc, in_maps, list(range(NCORES)), trace=False)
    acc = np.zeros((BS, D), np.float64)
    for c in range(NCORES):
        acc += res.results[c]["out"].astype(np.float64)
    acc += np.asarray(bo, np.float32)[None, :].astype(np.float64)
    return acc.reshape(B, S, D).astype(np.float32)
